# revision 32
# baseline (speedup 1.0000x reference)
"""Trainium2 Bass kernel: ViT-style multimodal transformer (12L, D=768, H=12).

Strategy: pure data parallel — 8 batch elements, one per NeuronCore.
Each core runs the full transformer on its [667, 768] token sequence.

Device layouts (per core):
  - residual x:   SBUF [128, 6, 768] fp32, token t = j*128 + p  (natural: t on partitions)
  - hT/QT/KT/OT:  SBUF [128, 6, 667] bf16, feature-major (transposed: d on partitions,
                  t on free dim) — the layout matmul wants for both lhsT and rhs roles.
  - attention:    S^T[s,t] = K_h Q_h^T computed per head with s on partitions, exp on
                  ScalarE (no max subtraction; logits are tiny), denominator obtained by
                  appending a ones-column to V in the AV matmul, normalization applied to
                  O' via a K=1 broadcast matmul + multiply.
  - all linear layers except V run in T-form (weights stationary, tokens streamed), so
    every weight block is DMA'd exactly once per layer via the HWDGE rings; proj/ffn2
    T-form outputs are transposed back on PE and accumulated into x, with LN statistics
    (bn_stats) computed eagerly per token tile inside the add-back.
Token order is permuted (attention is permutation-equivariant; positional embeddings are
baked into the additive base): [obs(392) | goal(196) | cls | pose | text(77)], so patch
embeddings land partition-aligned. cls lives at row 588 = (j=4, p=76).
"""

import numpy as np
import ml_dtypes

import concourse.bass as bass
import concourse.bacc as bacc_mod
import concourse.mybir as mybir
import concourse.tile as tile
from concourse.bass_utils import run_bass_kernel_spmd
from concourse.masks import make_identity

BF16 = mybir.dt.bfloat16
F32 = mybir.dt.float32
AF = mybir.ActivationFunctionType
ALU = mybir.AluOpType

L, H, D, HD = 12, 12, 768, 64
P, IMG, NP, HS = 16, 224, 196, 2
TBLK, VOCAB, POSE_DIM, OUT = 77, 96, 7, 7
B = 8
SEQ = 667          # 1 cls + 1 pose + 392 obs + 77 text + 196 goal
TPAD = 768         # padded token slots (6 partition tiles)
NT = 6             # token partition tiles
ND = 6             # feature partition tiles (768/128)
NF = 24            # ffn feature tiles (3072/128)
SCALE = float(D) ** -0.5
EPS = 1e-5

# token tiles (start, width)
TT = [(0, 128), (128, 128), (256, 128), (384, 128), (512, 128), (640, 27)]


def _chunks(total, cap=512):
    s = 0
    out = []
    while s < total:
        w = min(cap, total - s)
        out.append((s, w))
        s += w
    return out


CH_T = _chunks(SEQ)    # [(0,512),(512,155)]
CH_D = _chunks(D)      # [(0,512),(512,256)]

# Runtime knobs (test.py may flip these)
TRACE = False
TRACE_CORES = [0]
LAST_EXEC_NS = None
_CACHE = {}


def build_nc():
    nc = bacc_mod.Bacc()

    # ---- per-core data inputs ----
    base = nc.declare_dram_parameter("base", [TPAD, D], F32, isOutput=False)
    pobsT = nc.declare_dram_parameter("pobsT", [D, 392], BF16, isOutput=False)
    pgoalT = nc.declare_dram_parameter("pgoalT", [D, 204], BF16, isOutput=False)
    # ---- shared weights ----
    obs_w = nc.declare_dram_parameter("obs_w", [D, D], BF16, isOutput=False)
    goal_w = nc.declare_dram_parameter("goal_w", [D, D], BF16, isOutput=False)
    wq = nc.declare_dram_parameter("wq", [L, D, D], BF16, isOutput=False)
    wk = nc.declare_dram_parameter("wk", [L, D, D], BF16, isOutput=False)
    wv = nc.declare_dram_parameter("wv", [L, D, D], BF16, isOutput=False)
    pw = nc.declare_dram_parameter("pw", [L, D, D], BF16, isOutput=False)
    fw1 = nc.declare_dram_parameter("fw1", [L, D, 4 * D], BF16, isOutput=False)
    fw2 = nc.declare_dram_parameter("fw2", [L, 4 * D, D], BF16, isOutput=False)
    pb = nc.declare_dram_parameter("pb", [L, D], F32, isOutput=False)
    fb1 = nc.declare_dram_parameter("fb1", [L, 4 * D], F32, isOutput=False)
    fb2 = nc.declare_dram_parameter("fb2", [L, D], F32, isOutput=False)
    ln1g = nc.declare_dram_parameter("ln1g", [L, D], F32, isOutput=False)
    ln1b = nc.declare_dram_parameter("ln1b", [L, D], F32, isOutput=False)
    ln2g = nc.declare_dram_parameter("ln2g", [L, D], F32, isOutput=False)
    ln2b = nc.declare_dram_parameter("ln2b", [L, D], F32, isOutput=False)
    clsout = nc.declare_dram_parameter("clsout", [1, D], F32, isOutput=True)

    with tile.TileContext(nc) as tc:
        with (
            tc.tile_pool(name="singles", bufs=1) as singles,
            tc.tile_pool(name="lncols", bufs=4) as lncols,
            tc.tile_pool(name="wpool", bufs=2) as wpool,
            tc.tile_pool(name="f1pool", bufs=2) as f1pool,
            tc.tile_pool(name="f2pool", bufs=1) as f2pool,
            tc.tile_pool(name="epool", bufs=3) as epool,
            tc.tile_pool(name="rows", bufs=2) as rows,
            tc.tile_pool(name="hn", bufs=3) as hn,
            tc.tile_pool(name="upool", bufs=2) as upool,
            tc.tile_pool(name="stats", bufs=6) as stats,
            tc.tile_pool(name="rpool", bufs=2) as rpool,
            tc.tile_pool(name="pbig", bufs=4, space="PSUM") as pbig,
            tc.tile_pool(name="patt", bufs=4, space="PSUM") as patt,
        ):
            # ---------- persistent SBUF ----------
            ident = singles.tile([128, 128], BF16)
            make_identity(nc, ident)
            eps_sb = singles.tile([128, 1], F32)
            nc.vector.memset(eps_sb, EPS)
            ones_sb = singles.tile([1, 128], F32)
            nc.vector.memset(ones_sb, 1.0)

            x = singles.tile([128, NT, D], F32)            # residual stream
            hT = singles.tile([128, ND, SEQ], BF16)        # LN output, transposed
            QT = singles.tile([128, ND, SEQ], BF16)
            KT = singles.tile([128, ND, SEQ], BF16)
            vbuf = singles.tile([128, NT, H, HD + 1], BF16)  # V natural + ones col
            OT = singles.tile([128, ND, SEQ], BF16)        # attn out, transposed
            h3T = singles.tile([128, NF, SEQ], BF16)       # relu ffn hidden, transposed
            yT = singles.tile([128, ND, SEQ], BF16)        # proj/ffn2 out, transposed
            st_all = singles.tile([128, NT, 3, 6], F32)    # bn_stats staging
            mv_all = singles.tile([128, NT, 2], F32)       # mean/var per token tile

            nc.vector.memset(vbuf[:, :, :, HD:HD + 1], 1.0)

            # ---------- load residual base ----------
            nc.sync.dma_start(out=x[:], in_=base.rearrange("(j p) d -> p j d", p=128))

            # ---------- helpers ----------
            def emit_stats(ti, tw):
                xi = x[:tw, ti, :].rearrange("p (s c) -> p s c", s=3)
                for s in range(3):
                    nc.vector.bn_stats(out=st_all[:tw, ti, s, :], in_=xi[:, s, :])
                nc.vector.bn_aggr(out=mv_all[:tw, ti, :], in_=st_all[:tw, ti])

            # ---------- patch embeddings ----------
            def embed_add(psrcT, src_w, wtag, ptiles, dests):
                psrc = epool.tile([128, ND, psrcT.shape[1]], BF16, tag=f"p{wtag}",
                                  bufs=1)
                nc.sync.dma_start(out=psrc[:],
                                  in_=psrcT.rearrange("(kt kp) t -> kp kt t", kp=128))
                for gi in range(0, len(ptiles), 2):
                    grp = list(range(gi, min(gi + 2, len(ptiles))))
                    psums = {}
                    for t_i in grp:
                        psums[t_i] = [pbig.tile([128, w], F32, tag="pbig",
                                                name=f"ps{t_i}_{ci}")
                                      for ci, (s, w) in enumerate(CH_D)]
                    for k in range(ND):
                        wk_t = epool.tile([128, D], BF16, tag="ew", bufs=2)
                        nc.sync.dma_start(out=wk_t[:],
                                          in_=src_w[k * 128:(k + 1) * 128, :])
                        for t_i in grp:
                            c0, cw = ptiles[t_i]
                            for ci, (s, w) in enumerate(CH_D):
                                nc.tensor.matmul(
                                    psums[t_i][ci][:cw, :],
                                    lhsT=psrc[:, k, c0:c0 + cw],
                                    rhs=wk_t[:, s:s + w],
                                    start=(k == 0), stop=(k == ND - 1))
                    for t_i in grp:
                        c0, cw = ptiles[t_i]
                        r0, xj = dests[t_i]
                        for ci, (s, w) in enumerate(CH_D):
                            nc.vector.tensor_add(out=x[r0:r0 + cw, xj, s:s + w],
                                                 in0=x[r0:r0 + cw, xj, s:s + w],
                                                 in1=psums[t_i][ci][:cw, :])

            embed_add(pobsT, obs_w, "o",
                      [(0, 128), (128, 128), (256, 128), (384, 8)],
                      [(0, 0), (0, 1), (0, 2), (0, 3)])
            embed_add(pgoalT, goal_w, "g",
                      [(0, 128), (128, 76)],
                      [(0, 3), (0, 4)])
            for ti, (t0, tw) in enumerate(TT):
                emit_stats(ti, tw)

            CH_TILES = [[0, 1, 2, 3], [4, 5]]

            def load_ln_cols(g_dram, b_dram, tag):
                gcol = lncols.tile([128, ND], F32, tag=f"g{tag}")
                bcol = lncols.tile([128, ND], F32, tag=f"b{tag}")
                nc.gpsimd.dma_start(out=gcol[:],
                                    in_=g_dram.rearrange("(t p) -> p t", p=128))
                nc.gpsimd.dma_start(out=bcol[:],
                                    in_=b_dram.rearrange("(t p) -> p t", p=128))
                return gcol, bcol

            def ln_tiles(tiles, gcol, bcol):
                """x -> hT for the given token tiles (stats already in mv_all)."""
                for ti in tiles:
                    t0, tw = TT[ti]
                    rstd = stats.tile([128, 1], F32, tag="rstd")
                    nc.scalar.activation(out=rstd[:tw], in_=mv_all[:tw, ti, 1:2],
                                         func=AF.Sqrt, bias=eps_sb[:tw], scale=1.0)
                    nc.vector.reciprocal(out=rstd[:tw], in_=rstd[:tw])
                    hnat = hn.tile([128, D], BF16, tag="hnat")
                    nc.vector.tensor_scalar(out=hnat[:tw], in0=x[:tw, ti, :],
                                            scalar1=mv_all[:tw, ti, 0:1],
                                            scalar2=rstd[:tw],
                                            op0=ALU.subtract, op1=ALU.mult)
                    pt6 = patt.tile([128, ND, 128], BF16, tag="patt")
                    for dj in range(ND):
                        nc.tensor.transpose(pt6[:, dj, :tw],
                                            hnat[:tw, dj * 128:(dj + 1) * 128],
                                            ident[:tw, :tw])
                    for dj in range(ND):
                        if dj % 2 == 0:
                            nc.scalar.activation(out=hT[:, dj, t0:t0 + tw],
                                                 in_=pt6[:, dj, :tw], func=AF.Identity,
                                                 scale=gcol[:, dj:dj + 1],
                                                 bias=bcol[:, dj:dj + 1])
                        else:
                            nc.vector.tensor_scalar(out=hT[:, dj, t0:t0 + tw],
                                                    in0=pt6[:, dj, :tw],
                                                    scalar1=gcol[:, dj:dj + 1],
                                                    scalar2=bcol[:, dj:dj + 1],
                                                    op0=ALU.mult, op1=ALU.add)

            def tform_chunk(w_sb, ci, n_cnt, out_sb, on0, k_cnt, src_sb,
                            bias_col=None, relu=False):
                c0, cw = CH_T[ci]
                for j in range(n_cnt):
                    seg = pbig.tile([128, cw], F32, tag="pbig", name=f"sg{ci}_{j}")
                    for k in range(k_cnt):
                        nc.tensor.matmul(seg[:, :],
                                         lhsT=w_sb[:, k, j * 128:(j + 1) * 128],
                                         rhs=src_sb[:, k, c0:c0 + cw],
                                         start=(k == 0), stop=(k == k_cnt - 1))
                    n_out = on0 + j
                    dst = out_sb[:, n_out, c0:c0 + cw]
                    on_act = (j + ci) % 2 == 0
                    if relu:
                        if on_act:
                            nc.scalar.activation(out=dst, in_=seg[:, :], func=AF.Relu,
                                                 bias=bias_col[:, n_out:n_out + 1],
                                                 scale=1.0)
                        else:
                            nc.vector.tensor_scalar(out=dst, in0=seg[:, :],
                                                    scalar1=bias_col[:, n_out:n_out + 1],
                                                    scalar2=0.0,
                                                    op0=ALU.add, op1=ALU.max)
                    elif bias_col is not None:
                        if on_act:
                            nc.scalar.activation(out=dst, in_=seg[:, :],
                                                 func=AF.Identity,
                                                 bias=bias_col[:, n_out:n_out + 1],
                                                 scale=1.0)
                        else:
                            nc.vector.tensor_scalar(out=dst, in0=seg[:, :],
                                                    scalar1=bias_col[:, n_out:n_out + 1],
                                                    scalar2=None, op0=ALU.add)
                    else:
                        if on_act:
                            nc.scalar.copy(out=dst, in_=seg[:, :])
                        else:
                            nc.vector.tensor_copy(out=dst, in_=seg[:, :])

            def resid_chunk(src_sb, ci, with_stats):
                """x += transpose(src_sb) for the token tiles of chunk ci."""
                for ti in CH_TILES[ci]:
                    t0, tw = TT[ti]
                    pt6 = patt.tile([128, D], BF16, tag="patt")
                    for dj in range(ND):
                        nc.tensor.transpose(pt6[:tw, dj * 128:(dj + 1) * 128],
                                            src_sb[:, dj, t0:t0 + tw], ident)
                    nc.vector.tensor_add(out=x[:tw, ti, :], in0=x[:tw, ti, :],
                                         in1=pt6[:tw, :])
                    if with_stats:
                        emit_stats(ti, tw)

            def tform_resid_ln(w_sb, n_cnt, k_cnt, src_sb, bias_col,
                               ln_cols, with_stats):
                """T-form linear -> yT, chunk-major; residual add-back and the
                following LN interleaved so their DVE/ACT chains hide under
                the other chunk's matmuls. ln_cols None => skip LN emission."""
                tform_chunk(w_sb, 0, n_cnt, yT, 0, k_cnt, src_sb, bias_col)
                resid_chunk(yT, 0, with_stats)
                tform_chunk(w_sb, 1, n_cnt, yT, 0, k_cnt, src_sb, bias_col)
                if ln_cols is not None:
                    ln_tiles(CH_TILES[0], *ln_cols)
                resid_chunk(yT, 1, with_stats)
                if ln_cols is not None:
                    ln_tiles(CH_TILES[1], *ln_cols)

            def load_w(w_dram, tag, pool, ksz, c0, cw):
                t = pool.tile([128, ksz, cw], BF16, tag=tag)
                nc.sync.dma_start(
                    out=t[:],
                    in_=w_dram.rearrange("(kt kp) n -> kp kt n", kp=128)[:, :, c0:c0 + cw])
                return t

            # ---------- transformer layers ----------
            ln1_cols = load_ln_cols(ln1g[0], ln1b[0], "1")
            ln_tiles(CH_TILES[0] + CH_TILES[1], *ln1_cols)
            for l in range(L):
                # prefetch the big ffn2 slab early (its buffer frees at the
                # end of the previous layer's ffn2)
                f2_sb = load_w(fw2[l], "f2", f2pool, NF, 0, D)

                with nc.named_scope("qk"):
                    wq_sb = load_w(wq[l], "w", wpool, ND, 0, D)
                    for ci in range(2):
                        tform_chunk(wq_sb, ci, ND, QT, 0, ND, hT)
                    wk_sb = load_w(wk[l], "w", wpool, ND, 0, D)
                    for ci in range(2):
                        tform_chunk(wk_sb, ci, ND, KT, 0, ND, hT)

                # V natural into vbuf (+ ones col preset)
                with nc.named_scope("v"):
                    wv_sb = load_w(wv[l], "w", wpool, ND, 0, D)
                    for gi in range(0, NT, 2):
                        grp = [g for g in range(gi, min(gi + 2, NT))]
                        psums = {}
                        for t_i in grp:
                            psums[t_i] = [pbig.tile([128, w], F32, tag="pbig",
                                                    name=f"psv{t_i}_{ci}")
                                          for ci, (s, w) in enumerate(CH_D)]
                        for k in range(ND):
                            for t_i in grp:
                                t0, tw = TT[t_i]
                                for ci, (s, w) in enumerate(CH_D):
                                    nc.tensor.matmul(psums[t_i][ci][:tw, :],
                                                     lhsT=hT[:, k, t0:t0 + tw],
                                                     rhs=wv_sb[:, k, s:s + w],
                                                     start=(k == 0), stop=(k == ND - 1))
                        for t_i in grp:
                            t0, tw = TT[t_i]
                            for ci, (s, w) in enumerate(CH_D):
                                h0, nh = s // HD, w // HD
                                vsrc = psums[t_i][ci][:tw, :].rearrange(
                                    "p (h d) -> p h d", h=nh)
                                if (t_i + ci) % 2 == 0:
                                    nc.vector.tensor_copy(
                                        out=vbuf[:tw, t_i, h0:h0 + nh, 0:HD], in_=vsrc)
                                else:
                                    nc.scalar.copy(
                                        out=vbuf[:tw, t_i, h0:h0 + nh, 0:HD], in_=vsrc)

                # attention per head
                def emit_ST(h):
                    j, r = h // 2, (h % 2) * 64
                    u = upool.tile([128, NT, SEQ], BF16, tag="U")
                    for s_i, (s0, sw) in enumerate(TT):
                        for ci, (c, w) in enumerate(CH_T):
                            ps = patt.tile([128, 512], F32, tag="patt",
                                           name=f"pst{ci}")
                            nc.tensor.matmul(ps[:sw, :w],
                                             lhsT=KT[r:r + 64, j, s0:s0 + sw],
                                             rhs=QT[r:r + 64, j, c:c + w],
                                             start=True, stop=True)
                            nc.scalar.activation(out=u[:sw, s_i, c:c + w],
                                                 in_=ps[:sw, :w],
                                                 func=AF.Exp, scale=SCALE)
                    return u

                def emit_AV(h, u):
                    j, r = h // 2, (h % 2) * 64
                    po = [pbig.tile([128, w], F32, tag="pbig", name=f"po{ci}")
                          for ci, (c, w) in enumerate(CH_T)]
                    for s_i, (s0, sw) in enumerate(TT):
                        for ci, (c, w) in enumerate(CH_T):
                            nc.tensor.matmul(po[ci][:HD + 1, :],
                                             lhsT=vbuf[:sw, s_i, h, :],
                                             rhs=u[:sw, s_i, c:c + w],
                                             start=(s_i == 0), stop=(s_i == NT - 1))
                    rb = rpool.tile([1, SEQ], F32, tag="rb")
                    for ci, (c, w) in enumerate(CH_T):
                        nc.vector.reciprocal(out=rb[0:1, c:c + w],
                                             in_=po[ci][HD:HD + 1, :])
                    for ci, (c, w) in enumerate(CH_T):
                        pbc = patt.tile([128, 512], F32, tag="patt", name=f"pbc{ci}")
                        nc.tensor.matmul(pbc[:HD, :w],
                                         lhsT=ones_sb[0:1, :HD],
                                         rhs=rb[0:1, c:c + w],
                                         start=True, stop=True)
                        # drain O' to SBUF, then scale in place (one PSUM
                        # operand per DVE op)
                        nc.vector.tensor_copy(out=OT[r:r + 64, j, c:c + w],
                                              in_=po[ci][:HD, :])
                        nc.vector.tensor_mul(out=OT[r:r + 64, j, c:c + w],
                                             in0=OT[r:r + 64, j, c:c + w],
                                             in1=pbc[:HD, :w])

                with nc.named_scope("attn"):
                    u_prev = emit_ST(0)
                    for h in range(1, H):
                        u_cur = emit_ST(h)
                        emit_AV(h - 1, u_prev)
                        u_prev = u_cur
                    emit_AV(H - 1, u_prev)

                # proj + residual + LN2 (interleaved, chunk-major)
                with nc.named_scope("proj"):
                    pw_sb = load_w(pw[l], "w", wpool, ND, 0, D)
                    pbcol = rows.tile([128, ND], F32, tag="pbc")
                    nc.gpsimd.dma_start(out=pbcol[:],
                                        in_=pb[l].rearrange("(t p) -> p t", p=128))
                    ln2_cols = load_ln_cols(ln2g[l], ln2b[l], "2")
                    tform_resid_ln(pw_sb, ND, ND, OT, pbcol, ln2_cols,
                                   with_stats=True)

                # FFN1: 4 slabs of 6 n-tiles each
                with nc.named_scope("ffn1"):
                    fb1col = rows.tile([128, NF], F32, tag="fb1")
                    nc.gpsimd.dma_start(out=fb1col[:],
                                        in_=fb1[l].rearrange("(t p) -> p t", p=128))
                    for sl in range(4):
                        f1_sb = load_w(fw1[l], "f1", f1pool, ND, sl * D, D)
                        for ci in range(2):
                            tform_chunk(f1_sb, ci, ND, h3T, sl * ND, ND, hT,
                                        bias_col=fb1col, relu=True)

                # FFN2 + residual + next-layer LN1 (interleaved, chunk-major)
                with nc.named_scope("ffn2"):
                    fb2col = rows.tile([128, ND], F32, tag="fb2")
                    nc.gpsimd.dma_start(out=fb2col[:],
                                        in_=fb2[l].rearrange("(t p) -> p t", p=128))
                    if l < L - 1:
                        ln1_cols = load_ln_cols(ln1g[l + 1], ln1b[l + 1], "1")
                        tform_resid_ln(f2_sb, ND, NF, h3T, fb2col, ln1_cols,
                                       with_stats=True)
                    else:
                        tform_resid_ln(f2_sb, ND, NF, h3T, fb2col, None,
                                       with_stats=False)

            # ---------- output: cls residual row (row 588 = j4, p76) ----------
            nc.sync.dma_start(out=clsout[:, :], in_=x[76:77, 4, :])

    nc.finalize()
    return nc


# ======================= host side =======================

def _sincos_pos(T, d):
    i = np.arange(T, dtype=np.float64)[:, None]
    j = np.arange(d, dtype=np.float64)[None, :]
    je = np.where(j % 2 == 0, j, j - 1)
    ang = i / np.power(10000.0, je / d)
    pe = np.where(j % 2 == 0, np.sin(ang), np.cos(ang))
    return pe.astype(np.float32)


def _patchify_stacked(img):
    b = img.shape[0]
    x = img.reshape(b, IMG // P, P, IMG // P, P, 3, HS)
    x = x.transpose(0, 1, 3, 6, 2, 4, 5)
    return x.reshape(b, NP * HS, P * P * 3)


def _patchify3(img):
    b = img.shape[0]
    x = img.reshape(b, IMG // P, P, IMG // P, P, 3)
    x = x.transpose(0, 1, 3, 2, 4, 5)
    return x.reshape(b, NP, P * P * 3)


def _layernorm_np(v, g, b, eps=1e-5):
    m = v.mean(axis=-1, keepdims=True)
    s = v.var(axis=-1, keepdims=True)
    return (v - m) / np.sqrt(s + eps) * g + b


PERM = np.concatenate([np.arange(2, 394), np.arange(471, 667),
                       np.array([0, 1]), np.arange(394, 471)])


def kernel(**inputs):
    global LAST_EXEC_NS
    f32 = lambda k: np.asarray(inputs[k], dtype=np.float32)
    bf = lambda a: np.ascontiguousarray(np.asarray(a, dtype=np.float32)
                                        .astype(ml_dtypes.bfloat16))

    if "nc" not in _CACHE:
        _CACHE["nc"] = build_nc()
    nc = _CACHE["nc"]

    images = f32("images")
    goal_imgs = f32("goal_imgs")
    pose = f32("pose")
    txt = np.asarray(inputs["goals_txt"]).astype(np.int64)
    tok_emb = f32("tok_emb")

    # pose MLP (host, exact fp32 – 4.7 MFLOP)
    pose_tok = np.maximum(pose @ f32("pose_w1") + f32("pose_b1"), 0.0) \
        @ f32("pose_w2") + f32("pose_b2")                       # [B, D]

    pos = _sincos_pos(SEQ, D)                                    # [667, D]
    content = np.zeros((B, SEQ, D), np.float32)
    content[:, 0, :] = f32("cls_tok")[0, 0]
    content[:, 1, :] = pose_tok
    content[:, 2:394, :] = f32("obs_b")
    content[:, 394:471, :] = tok_emb[txt]
    content[:, 471:667, :] = f32("goal_b")
    base = (content + pos[None])[:, PERM, :]                     # permuted
    base_pad = np.zeros((B, TPAD, D), np.float32)
    base_pad[:, :SEQ, :] = base

    p_obs = _patchify_stacked(images)                            # [B, 392, 768]
    p_goal = _patchify3(goal_imgs)                               # [B, 196, 768]
    pobsT = bf(p_obs.transpose(0, 2, 1))                         # [B, 768, 392]
    pgoalT_np = np.zeros((B, D, 204), np.float32)
    pgoalT_np[:, :, 8:] = p_goal.transpose(0, 2, 1)
    pgoalT = bf(pgoalT_np)

    shared = {
        "obs_w": bf(f32("obs_w")), "goal_w": bf(f32("goal_w")),
        "wq": bf(f32("wq")), "wk": bf(f32("wk")), "wv": bf(f32("wv")),
        "pw": bf(f32("proj_w")), "fw1": bf(f32("ff_w1")), "fw2": bf(f32("ff_w2")),
        "pb": f32("proj_b"), "fb1": f32("ff_b1"), "fb2": f32("ff_b2"),
        "ln1g": f32("ln1_g"), "ln1b": f32("ln1_b"),
        "ln2g": f32("ln2_g"), "ln2b": f32("ln2_b"),
    }
    in_maps = []
    for b in range(B):
        m = dict(shared)
        m["base"] = np.ascontiguousarray(base_pad[b])
        m["pobsT"] = np.ascontiguousarray(pobsT[b])
        m["pgoalT"] = np.ascontiguousarray(pgoalT[b])
        in_maps.append(m)

    res = run_bass_kernel_spmd(nc, in_maps, list(range(B)), trace=TRACE,
                               trace_cores=TRACE_CORES if TRACE else None)
    LAST_EXEC_NS = res.exec_time_ns

    cls = np.stack([np.asarray(res.results[b]["clsout"][0], np.float32)
                    for b in range(B)])                          # [B, D]
    h = _layernorm_np(cls, f32("lnf_g"), f32("lnf_b"))
    h = _layernorm_np(h, f32("hln_g"), f32("hln_b"))
    out = h @ f32("head_w") + f32("head_b")
    return out.astype(np.float32)


# revision 33
# speedup vs baseline: 1.0129x; 1.0129x over previous
"""Trainium2 Bass kernel: ViT-style multimodal transformer (12L, D=768, H=12).

Strategy: pure data parallel — 8 batch elements, one per NeuronCore.
Each core runs the full transformer on its [667, 768] token sequence.

Device layouts (per core):
  - residual x:   SBUF [128, 6, 768] fp32, token t = j*128 + p  (natural: t on partitions)
  - hT/QT/KT/OT:  SBUF [128, 6, 667] bf16, feature-major (transposed: d on partitions,
                  t on free dim) — the layout matmul wants for both lhsT and rhs roles.
  - attention:    S^T[s,t] = K_h Q_h^T computed per head with s on partitions, exp on
                  ScalarE (no max subtraction; logits are tiny), denominator obtained by
                  appending a ones-column to V in the AV matmul, normalization applied to
                  O' via a K=1 broadcast matmul + multiply.
  - all linear layers except V run in T-form (weights stationary, tokens streamed), so
    every weight block is DMA'd exactly once per layer via the HWDGE rings; proj/ffn2
    T-form outputs are transposed back on PE and accumulated into x, with LN statistics
    (bn_stats) computed eagerly per token tile inside the add-back.
Token order is permuted (attention is permutation-equivariant; positional embeddings are
baked into the additive base): [obs(392) | goal(196) | cls | pose | text(77)], so patch
embeddings land partition-aligned. cls lives at row 588 = (j=4, p=76).
"""

import numpy as np
import ml_dtypes

import concourse.bass as bass
import concourse.bacc as bacc_mod
import concourse.mybir as mybir
import concourse.tile as tile
from concourse.bass_utils import run_bass_kernel_spmd
from concourse.masks import make_identity

BF16 = mybir.dt.bfloat16
F32 = mybir.dt.float32
AF = mybir.ActivationFunctionType
ALU = mybir.AluOpType

L, H, D, HD = 12, 12, 768, 64
P, IMG, NP, HS = 16, 224, 196, 2
TBLK, VOCAB, POSE_DIM, OUT = 77, 96, 7, 7
B = 8
SEQ = 667          # 1 cls + 1 pose + 392 obs + 77 text + 196 goal
TPAD = 768         # padded token slots (6 partition tiles)
NT = 6             # token partition tiles
ND = 6             # feature partition tiles (768/128)
NF = 24            # ffn feature tiles (3072/128)
SCALE = float(D) ** -0.5
EPS = 1e-5

# token tiles (start, width)
TT = [(0, 128), (128, 128), (256, 128), (384, 128), (512, 128), (640, 27)]


def _chunks(total, cap=512):
    s = 0
    out = []
    while s < total:
        w = min(cap, total - s)
        out.append((s, w))
        s += w
    return out


CH_T = _chunks(SEQ)    # [(0,512),(512,155)]
CH_D = _chunks(D)      # [(0,512),(512,256)]

# Runtime knobs (test.py may flip these)
TRACE = False
TRACE_CORES = [0]
LAST_EXEC_NS = None
_CACHE = {}


def build_nc():
    nc = bacc_mod.Bacc()

    # ---- per-core data inputs ----
    base = nc.declare_dram_parameter("base", [TPAD, D], F32, isOutput=False)
    pobsT = nc.declare_dram_parameter("pobsT", [D, 392], BF16, isOutput=False)
    pgoalT = nc.declare_dram_parameter("pgoalT", [D, 204], BF16, isOutput=False)
    # ---- shared weights ----
    obs_w = nc.declare_dram_parameter("obs_w", [D, D], BF16, isOutput=False)
    goal_w = nc.declare_dram_parameter("goal_w", [D, D], BF16, isOutput=False)
    wq = nc.declare_dram_parameter("wq", [L, D, D], BF16, isOutput=False)
    wk = nc.declare_dram_parameter("wk", [L, D, D], BF16, isOutput=False)
    wv = nc.declare_dram_parameter("wv", [L, D, D], BF16, isOutput=False)
    pw = nc.declare_dram_parameter("pw", [L, D, D], BF16, isOutput=False)
    fw1 = nc.declare_dram_parameter("fw1", [L, D, 4 * D], BF16, isOutput=False)
    fw2 = nc.declare_dram_parameter("fw2", [L, 4 * D, D], BF16, isOutput=False)
    pb = nc.declare_dram_parameter("pb", [L, D], F32, isOutput=False)
    fb1 = nc.declare_dram_parameter("fb1", [L, 4 * D], F32, isOutput=False)
    fb2 = nc.declare_dram_parameter("fb2", [L, D], F32, isOutput=False)
    ln1g = nc.declare_dram_parameter("ln1g", [L, D], F32, isOutput=False)
    ln1b = nc.declare_dram_parameter("ln1b", [L, D], F32, isOutput=False)
    ln2g = nc.declare_dram_parameter("ln2g", [L, D], F32, isOutput=False)
    ln2b = nc.declare_dram_parameter("ln2b", [L, D], F32, isOutput=False)
    clsout = nc.declare_dram_parameter("clsout", [1, D], F32, isOutput=True)

    with tile.TileContext(nc) as tc:
        with (
            tc.tile_pool(name="singles", bufs=1) as singles,
            tc.tile_pool(name="lncols", bufs=4) as lncols,
            tc.tile_pool(name="wpool", bufs=2) as wpool,
            tc.tile_pool(name="f1pool", bufs=2) as f1pool,
            tc.tile_pool(name="f2pool", bufs=1) as f2pool,
            tc.tile_pool(name="epool", bufs=3) as epool,
            tc.tile_pool(name="rows", bufs=2) as rows,
            tc.tile_pool(name="hn", bufs=3) as hn,
            tc.tile_pool(name="upool", bufs=2) as upool,
            tc.tile_pool(name="stats", bufs=6) as stats,
            tc.tile_pool(name="rpool", bufs=2) as rpool,
            tc.tile_pool(name="pbig", bufs=4, space="PSUM") as pbig,
            tc.tile_pool(name="patt", bufs=4, space="PSUM") as patt,
        ):
            # ---------- persistent SBUF ----------
            ident = singles.tile([128, 128], BF16)
            make_identity(nc, ident)
            eps_sb = singles.tile([128, 1], F32)
            nc.vector.memset(eps_sb, EPS)
            ones_sb = singles.tile([1, 128], F32)
            nc.vector.memset(ones_sb, 1.0)

            x = singles.tile([128, NT, D], F32)            # residual stream
            hT = singles.tile([128, ND, SEQ], BF16)        # LN output, transposed
            QT = singles.tile([128, ND, SEQ], BF16)
            KT = singles.tile([128, ND, SEQ], BF16)
            vbuf = singles.tile([128, NT, H, HD + 1], BF16)  # V natural + ones col
            OT = singles.tile([128, ND, SEQ], BF16)        # attn out, transposed
            h3T = singles.tile([128, NF, SEQ], BF16)       # relu ffn hidden, transposed
            yT = singles.tile([128, ND, SEQ], BF16)        # proj/ffn2 out, transposed
            st_all = singles.tile([128, NT, 3, 6], F32)    # bn_stats staging
            mv_all = singles.tile([128, NT, 2], F32)       # mean/var per token tile

            nc.vector.memset(vbuf[:, :, :, HD:HD + 1], 1.0)

            # ---------- load residual base ----------
            nc.sync.dma_start(out=x[:], in_=base.rearrange("(j p) d -> p j d", p=128))

            # ---------- helpers ----------
            def emit_stats(ti, tw):
                xi = x[:tw, ti, :].rearrange("p (s c) -> p s c", s=3)
                for s in range(3):
                    nc.vector.bn_stats(out=st_all[:tw, ti, s, :], in_=xi[:, s, :])
                nc.vector.bn_aggr(out=mv_all[:tw, ti, :], in_=st_all[:tw, ti])

            # ---------- patch embeddings ----------
            def embed_add(psrcT, src_w, wtag, ptiles, dests):
                psrc = epool.tile([128, ND, psrcT.shape[1]], BF16, tag=f"p{wtag}",
                                  bufs=1)
                nc.sync.dma_start(out=psrc[:],
                                  in_=psrcT.rearrange("(kt kp) t -> kp kt t", kp=128))
                # whole embed weight matrix in one DMA (f1pool slots are idle
                # until layer-0 ffn1)
                ew = f1pool.tile([128, ND, D], BF16, tag="f1")
                nc.sync.dma_start(
                    out=ew[:], in_=src_w.rearrange("(kt kp) n -> kp kt n", kp=128))
                for gi in range(0, len(ptiles), 2):
                    grp = list(range(gi, min(gi + 2, len(ptiles))))
                    psums = {}
                    for t_i in grp:
                        psums[t_i] = [pbig.tile([128, w], F32, tag="pbig",
                                                name=f"ps{t_i}_{ci}")
                                      for ci, (s, w) in enumerate(CH_D)]
                    for k in range(ND):
                        for t_i in grp:
                            c0, cw = ptiles[t_i]
                            for ci, (s, w) in enumerate(CH_D):
                                nc.tensor.matmul(
                                    psums[t_i][ci][:cw, :],
                                    lhsT=psrc[:, k, c0:c0 + cw],
                                    rhs=ew[:, k, s:s + w],
                                    start=(k == 0), stop=(k == ND - 1))
                    for t_i in grp:
                        c0, cw = ptiles[t_i]
                        r0, xj = dests[t_i]
                        for ci, (s, w) in enumerate(CH_D):
                            nc.vector.tensor_add(out=x[r0:r0 + cw, xj, s:s + w],
                                                 in0=x[r0:r0 + cw, xj, s:s + w],
                                                 in1=psums[t_i][ci][:cw, :])

            embed_add(pobsT, obs_w, "o",
                      [(0, 128), (128, 128), (256, 128), (384, 8)],
                      [(0, 0), (0, 1), (0, 2), (0, 3)])
            embed_add(pgoalT, goal_w, "g",
                      [(0, 128), (128, 76)],
                      [(0, 3), (0, 4)])
            for ti, (t0, tw) in enumerate(TT):
                emit_stats(ti, tw)

            CH_TILES = [[0, 1, 2, 3], [4, 5]]

            def load_ln_cols(g_dram, b_dram, tag):
                gcol = lncols.tile([128, ND], F32, tag=f"g{tag}")
                bcol = lncols.tile([128, ND], F32, tag=f"b{tag}")
                nc.gpsimd.dma_start(out=gcol[:],
                                    in_=g_dram.rearrange("(t p) -> p t", p=128))
                nc.gpsimd.dma_start(out=bcol[:],
                                    in_=b_dram.rearrange("(t p) -> p t", p=128))
                return gcol, bcol

            def ln_tiles(tiles, gcol, bcol):
                """x -> hT for the given token tiles (stats already in mv_all)."""
                for ti in tiles:
                    t0, tw = TT[ti]
                    rstd = stats.tile([128, 1], F32, tag="rstd")
                    nc.scalar.activation(out=rstd[:tw], in_=mv_all[:tw, ti, 1:2],
                                         func=AF.Sqrt, bias=eps_sb[:tw], scale=1.0)
                    nc.vector.reciprocal(out=rstd[:tw], in_=rstd[:tw])
                    hnat = hn.tile([128, D], BF16, tag="hnat")
                    nc.vector.tensor_scalar(out=hnat[:tw], in0=x[:tw, ti, :],
                                            scalar1=mv_all[:tw, ti, 0:1],
                                            scalar2=rstd[:tw],
                                            op0=ALU.subtract, op1=ALU.mult)
                    pt6 = patt.tile([128, ND, 128], BF16, tag="patt")
                    for dj in range(ND):
                        nc.tensor.transpose(pt6[:, dj, :tw],
                                            hnat[:tw, dj * 128:(dj + 1) * 128],
                                            ident[:tw, :tw])
                    for dj in range(ND):
                        if dj % 2 == 0:
                            nc.scalar.activation(out=hT[:, dj, t0:t0 + tw],
                                                 in_=pt6[:, dj, :tw], func=AF.Identity,
                                                 scale=gcol[:, dj:dj + 1],
                                                 bias=bcol[:, dj:dj + 1])
                        else:
                            nc.vector.tensor_scalar(out=hT[:, dj, t0:t0 + tw],
                                                    in0=pt6[:, dj, :tw],
                                                    scalar1=gcol[:, dj:dj + 1],
                                                    scalar2=bcol[:, dj:dj + 1],
                                                    op0=ALU.mult, op1=ALU.add)

            def tform_chunk(w_sb, ci, n_cnt, out_sb, on0, k_cnt, src_sb,
                            bias_col=None, relu=False):
                c0, cw = CH_T[ci]
                for j in range(n_cnt):
                    seg = pbig.tile([128, cw], F32, tag="pbig", name=f"sg{ci}_{j}")
                    for k in range(k_cnt):
                        nc.tensor.matmul(seg[:, :],
                                         lhsT=w_sb[:, k, j * 128:(j + 1) * 128],
                                         rhs=src_sb[:, k, c0:c0 + cw],
                                         start=(k == 0), stop=(k == k_cnt - 1))
                    n_out = on0 + j
                    dst = out_sb[:, n_out, c0:c0 + cw]
                    on_act = (j + ci) % 2 == 0
                    if relu:
                        if on_act:
                            nc.scalar.activation(out=dst, in_=seg[:, :], func=AF.Relu,
                                                 bias=bias_col[:, n_out:n_out + 1],
                                                 scale=1.0)
                        else:
                            nc.vector.tensor_scalar(out=dst, in0=seg[:, :],
                                                    scalar1=bias_col[:, n_out:n_out + 1],
                                                    scalar2=0.0,
                                                    op0=ALU.add, op1=ALU.max)
                    elif bias_col is not None:
                        if on_act:
                            nc.scalar.activation(out=dst, in_=seg[:, :],
                                                 func=AF.Identity,
                                                 bias=bias_col[:, n_out:n_out + 1],
                                                 scale=1.0)
                        else:
                            nc.vector.tensor_scalar(out=dst, in0=seg[:, :],
                                                    scalar1=bias_col[:, n_out:n_out + 1],
                                                    scalar2=None, op0=ALU.add)
                    else:
                        if on_act:
                            nc.scalar.copy(out=dst, in_=seg[:, :])
                        else:
                            nc.vector.tensor_copy(out=dst, in_=seg[:, :])

            def resid_chunk(src_sb, ci, with_stats):
                """x += transpose(src_sb) for the token tiles of chunk ci."""
                for ti in CH_TILES[ci]:
                    t0, tw = TT[ti]
                    pt6 = patt.tile([128, D], BF16, tag="patt")
                    for dj in range(ND):
                        nc.tensor.transpose(pt6[:tw, dj * 128:(dj + 1) * 128],
                                            src_sb[:, dj, t0:t0 + tw], ident)
                    nc.vector.tensor_add(out=x[:tw, ti, :], in0=x[:tw, ti, :],
                                         in1=pt6[:tw, :])
                    if with_stats:
                        emit_stats(ti, tw)

            def tform_resid_ln(w_sb, n_cnt, k_cnt, src_sb, bias_col,
                               ln_cols, with_stats):
                """T-form linear -> yT, chunk-major; residual add-back and the
                following LN interleaved so their DVE/ACT chains hide under
                the other chunk's matmuls. ln_cols None => skip LN emission."""
                tform_chunk(w_sb, 0, n_cnt, yT, 0, k_cnt, src_sb, bias_col)
                resid_chunk(yT, 0, with_stats)
                tform_chunk(w_sb, 1, n_cnt, yT, 0, k_cnt, src_sb, bias_col)
                if ln_cols is not None:
                    ln_tiles(CH_TILES[0], *ln_cols)
                resid_chunk(yT, 1, with_stats)
                if ln_cols is not None:
                    ln_tiles(CH_TILES[1], *ln_cols)

            def load_w(w_dram, tag, pool, ksz, c0, cw):
                t = pool.tile([128, ksz, cw], BF16, tag=tag)
                nc.sync.dma_start(
                    out=t[:],
                    in_=w_dram.rearrange("(kt kp) n -> kp kt n", kp=128)[:, :, c0:c0 + cw])
                return t

            # ---------- transformer layers ----------
            ln1_cols = load_ln_cols(ln1g[0], ln1b[0], "1")
            ln_tiles(CH_TILES[0] + CH_TILES[1], *ln1_cols)
            for l in range(L):
                # prefetch the big ffn2 slab early (its buffer frees at the
                # end of the previous layer's ffn2)
                f2_sb = load_w(fw2[l], "f2", f2pool, NF, 0, D)

                with nc.named_scope("qk"):
                    wq_sb = load_w(wq[l], "w", wpool, ND, 0, D)
                    for ci in range(2):
                        tform_chunk(wq_sb, ci, ND, QT, 0, ND, hT)
                    wk_sb = load_w(wk[l], "w", wpool, ND, 0, D)
                    for ci in range(2):
                        tform_chunk(wk_sb, ci, ND, KT, 0, ND, hT)

                # V natural into vbuf (+ ones col preset)
                with nc.named_scope("v"):
                    wv_sb = load_w(wv[l], "w", wpool, ND, 0, D)
                    for gi in range(0, NT, 2):
                        grp = [g for g in range(gi, min(gi + 2, NT))]
                        psums = {}
                        for t_i in grp:
                            psums[t_i] = [pbig.tile([128, w], F32, tag="pbig",
                                                    name=f"psv{t_i}_{ci}")
                                          for ci, (s, w) in enumerate(CH_D)]
                        for k in range(ND):
                            for t_i in grp:
                                t0, tw = TT[t_i]
                                for ci, (s, w) in enumerate(CH_D):
                                    nc.tensor.matmul(psums[t_i][ci][:tw, :],
                                                     lhsT=hT[:, k, t0:t0 + tw],
                                                     rhs=wv_sb[:, k, s:s + w],
                                                     start=(k == 0), stop=(k == ND - 1))
                        for t_i in grp:
                            t0, tw = TT[t_i]
                            for ci, (s, w) in enumerate(CH_D):
                                h0, nh = s // HD, w // HD
                                vsrc = psums[t_i][ci][:tw, :].rearrange(
                                    "p (h d) -> p h d", h=nh)
                                if (t_i + ci) % 2 == 0:
                                    nc.vector.tensor_copy(
                                        out=vbuf[:tw, t_i, h0:h0 + nh, 0:HD], in_=vsrc)
                                else:
                                    nc.scalar.copy(
                                        out=vbuf[:tw, t_i, h0:h0 + nh, 0:HD], in_=vsrc)

                # attention per head
                def emit_ST(h):
                    j, r = h // 2, (h % 2) * 64
                    u = upool.tile([128, NT, SEQ], BF16, tag="U")
                    for s_i, (s0, sw) in enumerate(TT):
                        for ci, (c, w) in enumerate(CH_T):
                            ps = patt.tile([128, 512], F32, tag="patt",
                                           name=f"pst{ci}")
                            nc.tensor.matmul(ps[:sw, :w],
                                             lhsT=KT[r:r + 64, j, s0:s0 + sw],
                                             rhs=QT[r:r + 64, j, c:c + w],
                                             start=True, stop=True)
                            nc.scalar.activation(out=u[:sw, s_i, c:c + w],
                                                 in_=ps[:sw, :w],
                                                 func=AF.Exp, scale=SCALE)
                    return u

                def emit_AV(h, u):
                    j, r = h // 2, (h % 2) * 64
                    po = [pbig.tile([128, w], F32, tag="pbig", name=f"po{ci}")
                          for ci, (c, w) in enumerate(CH_T)]
                    for s_i, (s0, sw) in enumerate(TT):
                        for ci, (c, w) in enumerate(CH_T):
                            nc.tensor.matmul(po[ci][:HD + 1, :],
                                             lhsT=vbuf[:sw, s_i, h, :],
                                             rhs=u[:sw, s_i, c:c + w],
                                             start=(s_i == 0), stop=(s_i == NT - 1))
                    rb = rpool.tile([1, SEQ], F32, tag="rb")
                    for ci, (c, w) in enumerate(CH_T):
                        nc.vector.reciprocal(out=rb[0:1, c:c + w],
                                             in_=po[ci][HD:HD + 1, :])
                    for ci, (c, w) in enumerate(CH_T):
                        pbc = patt.tile([128, 512], F32, tag="patt", name=f"pbc{ci}")
                        nc.tensor.matmul(pbc[:HD, :w],
                                         lhsT=ones_sb[0:1, :HD],
                                         rhs=rb[0:1, c:c + w],
                                         start=True, stop=True)
                        # drain O' to SBUF, then scale in place (one PSUM
                        # operand per DVE op)
                        nc.vector.tensor_copy(out=OT[r:r + 64, j, c:c + w],
                                              in_=po[ci][:HD, :])
                        nc.vector.tensor_mul(out=OT[r:r + 64, j, c:c + w],
                                             in0=OT[r:r + 64, j, c:c + w],
                                             in1=pbc[:HD, :w])

                with nc.named_scope("attn"):
                    u_prev = emit_ST(0)
                    for h in range(1, H):
                        u_cur = emit_ST(h)
                        emit_AV(h - 1, u_prev)
                        u_prev = u_cur
                    emit_AV(H - 1, u_prev)

                # proj + residual + LN2 (interleaved, chunk-major)
                with nc.named_scope("proj"):
                    pw_sb = load_w(pw[l], "w", wpool, ND, 0, D)
                    pbcol = rows.tile([128, ND], F32, tag="pbc")
                    nc.gpsimd.dma_start(out=pbcol[:],
                                        in_=pb[l].rearrange("(t p) -> p t", p=128))
                    ln2_cols = load_ln_cols(ln2g[l], ln2b[l], "2")
                    tform_resid_ln(pw_sb, ND, ND, OT, pbcol, ln2_cols,
                                   with_stats=True)

                # FFN1: 4 slabs of 6 n-tiles each
                with nc.named_scope("ffn1"):
                    fb1col = rows.tile([128, NF], F32, tag="fb1")
                    nc.gpsimd.dma_start(out=fb1col[:],
                                        in_=fb1[l].rearrange("(t p) -> p t", p=128))
                    for sl in range(4):
                        f1_sb = load_w(fw1[l], "f1", f1pool, ND, sl * D, D)
                        for ci in range(2):
                            tform_chunk(f1_sb, ci, ND, h3T, sl * ND, ND, hT,
                                        bias_col=fb1col, relu=True)

                # FFN2 + residual + next-layer LN1 (interleaved, chunk-major)
                with nc.named_scope("ffn2"):
                    fb2col = rows.tile([128, ND], F32, tag="fb2")
                    nc.gpsimd.dma_start(out=fb2col[:],
                                        in_=fb2[l].rearrange("(t p) -> p t", p=128))
                    if l < L - 1:
                        ln1_cols = load_ln_cols(ln1g[l + 1], ln1b[l + 1], "1")
                        tform_resid_ln(f2_sb, ND, NF, h3T, fb2col, ln1_cols,
                                       with_stats=True)
                    else:
                        tform_resid_ln(f2_sb, ND, NF, h3T, fb2col, None,
                                       with_stats=False)

            # ---------- output: cls residual row (row 588 = j4, p76) ----------
            nc.sync.dma_start(out=clsout[:, :], in_=x[76:77, 4, :])

    nc.finalize()
    return nc


# ======================= host side =======================

def _sincos_pos(T, d):
    i = np.arange(T, dtype=np.float64)[:, None]
    j = np.arange(d, dtype=np.float64)[None, :]
    je = np.where(j % 2 == 0, j, j - 1)
    ang = i / np.power(10000.0, je / d)
    pe = np.where(j % 2 == 0, np.sin(ang), np.cos(ang))
    return pe.astype(np.float32)


def _patchify_stacked(img):
    b = img.shape[0]
    x = img.reshape(b, IMG // P, P, IMG // P, P, 3, HS)
    x = x.transpose(0, 1, 3, 6, 2, 4, 5)
    return x.reshape(b, NP * HS, P * P * 3)


def _patchify3(img):
    b = img.shape[0]
    x = img.reshape(b, IMG // P, P, IMG // P, P, 3)
    x = x.transpose(0, 1, 3, 2, 4, 5)
    return x.reshape(b, NP, P * P * 3)


def _layernorm_np(v, g, b, eps=1e-5):
    m = v.mean(axis=-1, keepdims=True)
    s = v.var(axis=-1, keepdims=True)
    return (v - m) / np.sqrt(s + eps) * g + b


PERM = np.concatenate([np.arange(2, 394), np.arange(471, 667),
                       np.array([0, 1]), np.arange(394, 471)])


def kernel(**inputs):
    global LAST_EXEC_NS
    f32 = lambda k: np.asarray(inputs[k], dtype=np.float32)
    bf = lambda a: np.ascontiguousarray(np.asarray(a, dtype=np.float32)
                                        .astype(ml_dtypes.bfloat16))

    if "nc" not in _CACHE:
        _CACHE["nc"] = build_nc()
    nc = _CACHE["nc"]

    images = f32("images")
    goal_imgs = f32("goal_imgs")
    pose = f32("pose")
    txt = np.asarray(inputs["goals_txt"]).astype(np.int64)
    tok_emb = f32("tok_emb")

    # pose MLP (host, exact fp32 – 4.7 MFLOP)
    pose_tok = np.maximum(pose @ f32("pose_w1") + f32("pose_b1"), 0.0) \
        @ f32("pose_w2") + f32("pose_b2")                       # [B, D]

    pos = _sincos_pos(SEQ, D)                                    # [667, D]
    content = np.zeros((B, SEQ, D), np.float32)
    content[:, 0, :] = f32("cls_tok")[0, 0]
    content[:, 1, :] = pose_tok
    content[:, 2:394, :] = f32("obs_b")
    content[:, 394:471, :] = tok_emb[txt]
    content[:, 471:667, :] = f32("goal_b")
    base = (content + pos[None])[:, PERM, :]                     # permuted
    base_pad = np.zeros((B, TPAD, D), np.float32)
    base_pad[:, :SEQ, :] = base

    p_obs = _patchify_stacked(images)                            # [B, 392, 768]
    p_goal = _patchify3(goal_imgs)                               # [B, 196, 768]
    pobsT = bf(p_obs.transpose(0, 2, 1))                         # [B, 768, 392]
    pgoalT_np = np.zeros((B, D, 204), np.float32)
    pgoalT_np[:, :, 8:] = p_goal.transpose(0, 2, 1)
    pgoalT = bf(pgoalT_np)

    shared = {
        "obs_w": bf(f32("obs_w")), "goal_w": bf(f32("goal_w")),
        "wq": bf(f32("wq")), "wk": bf(f32("wk")), "wv": bf(f32("wv")),
        "pw": bf(f32("proj_w")), "fw1": bf(f32("ff_w1")), "fw2": bf(f32("ff_w2")),
        "pb": f32("proj_b"), "fb1": f32("ff_b1"), "fb2": f32("ff_b2"),
        "ln1g": f32("ln1_g"), "ln1b": f32("ln1_b"),
        "ln2g": f32("ln2_g"), "ln2b": f32("ln2_b"),
    }
    in_maps = []
    for b in range(B):
        m = dict(shared)
        m["base"] = np.ascontiguousarray(base_pad[b])
        m["pobsT"] = np.ascontiguousarray(pobsT[b])
        m["pgoalT"] = np.ascontiguousarray(pgoalT[b])
        in_maps.append(m)

    res = run_bass_kernel_spmd(nc, in_maps, list(range(B)), trace=TRACE,
                               trace_cores=TRACE_CORES if TRACE else None)
    LAST_EXEC_NS = res.exec_time_ns

    cls = np.stack([np.asarray(res.results[b]["clsout"][0], np.float32)
                    for b in range(B)])                          # [B, D]
    h = _layernorm_np(cls, f32("lnf_g"), f32("lnf_b"))
    h = _layernorm_np(h, f32("hln_g"), f32("hln_b"))
    out = h @ f32("head_w") + f32("head_b")
    return out.astype(np.float32)


# revision 34
# speedup vs baseline: 1.0439x; 1.0306x over previous
"""Trainium2 Bass kernel: ViT-style multimodal transformer (12L, D=768, H=12).

Strategy: pure data parallel — 8 batch elements, one per NeuronCore.
Each core runs the full transformer on its [667, 768] token sequence.

Device layouts (per core):
  - residual x:   SBUF [128, 6, 768] fp32, token t = j*128 + p  (natural: t on partitions)
  - hT/QT/KT/OT:  SBUF [128, 6, 667] bf16, feature-major (transposed: d on partitions,
                  t on free dim) — the layout matmul wants for both lhsT and rhs roles.
  - attention:    S^T[s,t] = K_h Q_h^T computed per head with s on partitions, exp on
                  ScalarE (no max subtraction; logits are tiny), denominator obtained by
                  appending a ones-column to V in the AV matmul, normalization applied to
                  O' via a K=1 broadcast matmul + multiply.
  - all linear layers except V run in T-form (weights stationary, tokens streamed), so
    every weight block is DMA'd exactly once per layer via the HWDGE rings; proj/ffn2
    T-form outputs are transposed back on PE and accumulated into x, with LN statistics
    (bn_stats) computed eagerly per token tile inside the add-back.
Token order is permuted (attention is permutation-equivariant; positional embeddings are
baked into the additive base): [obs(392) | goal(196) | cls | pose | text(77)], so patch
embeddings land partition-aligned. cls lives at row 588 = (j=4, p=76).
"""

import numpy as np
import ml_dtypes

import concourse.bass as bass
import concourse.bacc as bacc_mod
import concourse.mybir as mybir
import concourse.tile as tile
from concourse.bass_utils import run_bass_kernel_spmd
from concourse.masks import make_identity

BF16 = mybir.dt.bfloat16
F32 = mybir.dt.float32
AF = mybir.ActivationFunctionType
ALU = mybir.AluOpType

L, H, D, HD = 12, 12, 768, 64
P, IMG, NP, HS = 16, 224, 196, 2
TBLK, VOCAB, POSE_DIM, OUT = 77, 96, 7, 7
B = 8
SEQ = 667          # 1 cls + 1 pose + 392 obs + 77 text + 196 goal
TPAD = 768         # padded token slots (6 partition tiles)
NT = 6             # token partition tiles
ND = 6             # feature partition tiles (768/128)
NF = 24            # ffn feature tiles (3072/128)
SCALE = float(D) ** -0.5
EPS = 1e-5

# token tiles (start, width)
TT = [(0, 128), (128, 128), (256, 128), (384, 128), (512, 128), (640, 27)]


def _chunks(total, cap=512):
    s = 0
    out = []
    while s < total:
        w = min(cap, total - s)
        out.append((s, w))
        s += w
    return out


CH_T = _chunks(SEQ)    # [(0,512),(512,155)]
CH_D = _chunks(D)      # [(0,512),(512,256)]

# Runtime knobs (test.py may flip these)
TRACE = False
TRACE_CORES = [0]
LAST_EXEC_NS = None
_CACHE = {}


def build_nc():
    nc = bacc_mod.Bacc()

    # ---- per-core data inputs ----
    base = nc.declare_dram_parameter("base", [TPAD, D], F32, isOutput=False)
    pobsT = nc.declare_dram_parameter("pobsT", [D, 392], BF16, isOutput=False)
    pgoalT = nc.declare_dram_parameter("pgoalT", [D, 204], BF16, isOutput=False)
    # ---- shared weights ----
    obs_w = nc.declare_dram_parameter("obs_w", [D, D], BF16, isOutput=False)
    goal_w = nc.declare_dram_parameter("goal_w", [D, D], BF16, isOutput=False)
    wq = nc.declare_dram_parameter("wq", [L, D, D], BF16, isOutput=False)
    wk = nc.declare_dram_parameter("wk", [L, D, D], BF16, isOutput=False)
    wv = nc.declare_dram_parameter("wv", [L, D, D], BF16, isOutput=False)
    pw = nc.declare_dram_parameter("pw", [L, D, D], BF16, isOutput=False)
    fw1 = nc.declare_dram_parameter("fw1", [L, D, 4 * D], BF16, isOutput=False)
    fw2 = nc.declare_dram_parameter("fw2", [L, 4 * D, D], BF16, isOutput=False)
    pb = nc.declare_dram_parameter("pb", [L, D], F32, isOutput=False)
    fb1 = nc.declare_dram_parameter("fb1", [L, 4 * D], F32, isOutput=False)
    fb2 = nc.declare_dram_parameter("fb2", [L, D], F32, isOutput=False)
    ln1g = nc.declare_dram_parameter("ln1g", [L, D], F32, isOutput=False)
    ln1b = nc.declare_dram_parameter("ln1b", [L, D], F32, isOutput=False)
    ln2g = nc.declare_dram_parameter("ln2g", [L, D], F32, isOutput=False)
    ln2b = nc.declare_dram_parameter("ln2b", [L, D], F32, isOutput=False)
    clsout = nc.declare_dram_parameter("clsout", [1, D], F32, isOutput=True)

    with tile.TileContext(nc) as tc:
        with (
            tc.tile_pool(name="singles", bufs=1) as singles,
            tc.tile_pool(name="lncols", bufs=4) as lncols,
            tc.tile_pool(name="wpool", bufs=2) as wpool,
            tc.tile_pool(name="f1pool", bufs=2) as f1pool,
            tc.tile_pool(name="f2pool", bufs=1) as f2pool,
            tc.tile_pool(name="epool", bufs=3) as epool,
            tc.tile_pool(name="rows", bufs=2) as rows,
            tc.tile_pool(name="hn", bufs=3) as hn,
            tc.tile_pool(name="upool", bufs=2) as upool,
            tc.tile_pool(name="stats", bufs=6) as stats,
            tc.tile_pool(name="rpool", bufs=2) as rpool,
            tc.tile_pool(name="pbig", bufs=4, space="PSUM") as pbig,
            tc.tile_pool(name="patt", bufs=4, space="PSUM") as patt,
        ):
            # ---------- persistent SBUF ----------
            ident = singles.tile([128, 128], BF16)
            make_identity(nc, ident)
            eps_sb = singles.tile([128, 1], F32)
            nc.vector.memset(eps_sb, EPS)
            ones_sb = singles.tile([1, 128], F32)
            nc.vector.memset(ones_sb, 1.0)

            x = singles.tile([128, NT, D], F32)            # residual stream
            hT = singles.tile([128, ND, SEQ], BF16)        # LN output, transposed
            QT = singles.tile([128, ND, SEQ], BF16)
            KT = singles.tile([128, ND, SEQ], BF16)
            vbuf = singles.tile([128, NT, H, HD + 1], BF16)  # V natural + ones col
            OT = singles.tile([128, ND, SEQ], BF16)        # attn out, transposed
            h3T = singles.tile([128, NF, SEQ], BF16)       # relu ffn hidden, transposed
            yT = singles.tile([128, ND, SEQ], BF16)        # proj/ffn2 out, transposed
            st_all = singles.tile([128, NT, 3, 6], F32)    # bn_stats staging
            mv_all = singles.tile([128, NT, 2], F32)       # mean/var per token tile

            nc.vector.memset(vbuf[:, :, :, HD:HD + 1], 1.0)

            # ---------- load residual base ----------
            nc.sync.dma_start(out=x[:], in_=base.rearrange("(j p) d -> p j d", p=128))

            # ---------- helpers ----------
            def emit_stats(ti, tw):
                xi = x[:tw, ti, :].rearrange("p (s c) -> p s c", s=3)
                for s in range(3):
                    nc.vector.bn_stats(out=st_all[:tw, ti, s, :], in_=xi[:, s, :])
                nc.vector.bn_aggr(out=mv_all[:tw, ti, :], in_=st_all[:tw, ti])

            # ---------- patch embeddings ----------
            def embed_add(psrcT, src_w, wtag, ptiles, dests):
                psrc = epool.tile([128, ND, psrcT.shape[1]], BF16, tag=f"p{wtag}",
                                  bufs=1)
                nc.sync.dma_start(out=psrc[:],
                                  in_=psrcT.rearrange("(kt kp) t -> kp kt t", kp=128))
                # whole embed weight matrix in one DMA (f1pool slots are idle
                # until layer-0 ffn1)
                ew = f1pool.tile([128, ND, D], BF16, tag="f1")
                nc.sync.dma_start(
                    out=ew[:], in_=src_w.rearrange("(kt kp) n -> kp kt n", kp=128))
                for gi in range(0, len(ptiles), 2):
                    grp = list(range(gi, min(gi + 2, len(ptiles))))
                    psums = {}
                    for t_i in grp:
                        psums[t_i] = [pbig.tile([128, w], F32, tag="pbig",
                                                name=f"ps{t_i}_{ci}")
                                      for ci, (s, w) in enumerate(CH_D)]
                    for k in range(ND):
                        for t_i in grp:
                            c0, cw = ptiles[t_i]
                            for ci, (s, w) in enumerate(CH_D):
                                nc.tensor.matmul(
                                    psums[t_i][ci][:cw, :],
                                    lhsT=psrc[:, k, c0:c0 + cw],
                                    rhs=ew[:, k, s:s + w],
                                    start=(k == 0), stop=(k == ND - 1))
                    for t_i in grp:
                        c0, cw = ptiles[t_i]
                        r0, xj = dests[t_i]
                        for ci, (s, w) in enumerate(CH_D):
                            nc.vector.tensor_add(out=x[r0:r0 + cw, xj, s:s + w],
                                                 in0=x[r0:r0 + cw, xj, s:s + w],
                                                 in1=psums[t_i][ci][:cw, :])

            embed_add(pobsT, obs_w, "o",
                      [(0, 128), (128, 128), (256, 128), (384, 8)],
                      [(0, 0), (0, 1), (0, 2), (0, 3)])
            embed_add(pgoalT, goal_w, "g",
                      [(0, 128), (128, 76)],
                      [(0, 3), (0, 4)])
            for ti, (t0, tw) in enumerate(TT):
                emit_stats(ti, tw)

            CH_TILES = [[0, 1, 2, 3], [4, 5]]

            def load_ln_cols(g_dram, b_dram, tag):
                gcol = lncols.tile([128, ND], F32, tag=f"g{tag}")
                bcol = lncols.tile([128, ND], F32, tag=f"b{tag}")
                nc.gpsimd.dma_start(out=gcol[:],
                                    in_=g_dram.rearrange("(t p) -> p t", p=128))
                nc.gpsimd.dma_start(out=bcol[:],
                                    in_=b_dram.rearrange("(t p) -> p t", p=128))
                return gcol, bcol

            def ln_tiles(tiles, gcol, bcol):
                """x -> hT for the given token tiles (stats already in mv_all)."""
                for ti in tiles:
                    t0, tw = TT[ti]
                    rstd = stats.tile([128, 1], F32, tag="rstd")
                    nc.scalar.activation(out=rstd[:tw], in_=mv_all[:tw, ti, 1:2],
                                         func=AF.Sqrt, bias=eps_sb[:tw], scale=1.0)
                    nc.vector.reciprocal(out=rstd[:tw], in_=rstd[:tw])
                    hnat = hn.tile([128, D], BF16, tag="hnat")
                    nc.vector.tensor_scalar(out=hnat[:tw], in0=x[:tw, ti, :],
                                            scalar1=mv_all[:tw, ti, 0:1],
                                            scalar2=rstd[:tw],
                                            op0=ALU.subtract, op1=ALU.mult)
                    pt6 = patt.tile([128, ND, 128], BF16, tag="patt")
                    for dj in range(ND):
                        nc.tensor.transpose(pt6[:, dj, :tw],
                                            hnat[:tw, dj * 128:(dj + 1) * 128],
                                            ident[:tw, :tw])
                    for dj in range(ND):
                        if dj % 2 == 0:
                            nc.scalar.activation(out=hT[:, dj, t0:t0 + tw],
                                                 in_=pt6[:, dj, :tw], func=AF.Identity,
                                                 scale=gcol[:, dj:dj + 1],
                                                 bias=bcol[:, dj:dj + 1])
                        else:
                            nc.vector.tensor_scalar(out=hT[:, dj, t0:t0 + tw],
                                                    in0=pt6[:, dj, :tw],
                                                    scalar1=gcol[:, dj:dj + 1],
                                                    scalar2=bcol[:, dj:dj + 1],
                                                    op0=ALU.mult, op1=ALU.add)

            def tform_chunk(w_sb, ci, n_cnt, out_sb, on0, k_cnt, src_sb,
                            bias_col=None, relu=False):
                c0, cw = CH_T[ci]
                for j in range(n_cnt):
                    seg = pbig.tile([128, cw], F32, tag="pbig", name=f"sg{ci}_{j}")
                    for k in range(k_cnt):
                        nc.tensor.matmul(seg[:, :],
                                         lhsT=w_sb[:, k, j * 128:(j + 1) * 128],
                                         rhs=src_sb[:, k, c0:c0 + cw],
                                         start=(k == 0), stop=(k == k_cnt - 1))
                    n_out = on0 + j
                    dst = out_sb[:, n_out, c0:c0 + cw]
                    on_act = (j + ci) % 2 == 0
                    if relu:
                        if on_act:
                            nc.scalar.activation(out=dst, in_=seg[:, :], func=AF.Relu,
                                                 bias=bias_col[:, n_out:n_out + 1],
                                                 scale=1.0)
                        else:
                            nc.vector.tensor_scalar(out=dst, in0=seg[:, :],
                                                    scalar1=bias_col[:, n_out:n_out + 1],
                                                    scalar2=0.0,
                                                    op0=ALU.add, op1=ALU.max)
                    elif bias_col is not None:
                        if on_act:
                            nc.scalar.activation(out=dst, in_=seg[:, :],
                                                 func=AF.Identity,
                                                 bias=bias_col[:, n_out:n_out + 1],
                                                 scale=1.0)
                        else:
                            nc.vector.tensor_scalar(out=dst, in0=seg[:, :],
                                                    scalar1=bias_col[:, n_out:n_out + 1],
                                                    scalar2=None, op0=ALU.add)
                    else:
                        if on_act:
                            nc.scalar.copy(out=dst, in_=seg[:, :])
                        else:
                            nc.vector.tensor_copy(out=dst, in_=seg[:, :])

            def resid_chunk(src_sb, ci, with_stats):
                """x += transpose(src_sb) for the token tiles of chunk ci."""
                for ti in CH_TILES[ci]:
                    t0, tw = TT[ti]
                    pt6 = patt.tile([128, D], BF16, tag="patt")
                    for dj in range(ND):
                        nc.tensor.transpose(pt6[:tw, dj * 128:(dj + 1) * 128],
                                            src_sb[:, dj, t0:t0 + tw], ident)
                    nc.vector.tensor_add(out=x[:tw, ti, :], in0=x[:tw, ti, :],
                                         in1=pt6[:tw, :])
                    if with_stats:
                        emit_stats(ti, tw)

            def tform_resid_ln(w_sb, n_cnt, k_cnt, src_sb, bias_col,
                               ln_cols, with_stats):
                """T-form linear -> yT, chunk-major; residual add-back and the
                following LN interleaved so their DVE/ACT chains hide under
                the other chunk's matmuls. ln_cols None => skip LN emission."""
                tform_chunk(w_sb, 0, n_cnt, yT, 0, k_cnt, src_sb, bias_col)
                resid_chunk(yT, 0, with_stats)
                tform_chunk(w_sb, 1, n_cnt, yT, 0, k_cnt, src_sb, bias_col)
                if ln_cols is not None:
                    ln_tiles(CH_TILES[0], *ln_cols)
                resid_chunk(yT, 1, with_stats)
                if ln_cols is not None:
                    ln_tiles(CH_TILES[1], *ln_cols)

            def load_w(w_dram, tag, pool, ksz, c0, cw):
                t = pool.tile([128, ksz, cw], BF16, tag=tag)
                nc.sync.dma_start(
                    out=t[:],
                    in_=w_dram.rearrange("(kt kp) n -> kp kt n", kp=128)[:, :, c0:c0 + cw])
                return t

            # ---------- transformer layers ----------
            ln1_cols = load_ln_cols(ln1g[0], ln1b[0], "1")
            ln_tiles(CH_TILES[0] + CH_TILES[1], *ln1_cols)
            for l in range(L):
                # prefetch the big ffn2 slab early (its buffer frees at the
                # end of the previous layer's ffn2)
                f2_sb = load_w(fw2[l], "f2", f2pool, NF, 0, D)

                with nc.named_scope("qk"):
                    wq_sb = load_w(wq[l], "w", wpool, ND, 0, D)
                    for ci in range(2):
                        tform_chunk(wq_sb, ci, ND, QT, 0, ND, hT)
                    wk_sb = load_w(wk[l], "w", wpool, ND, 0, D)
                    for ci in range(2):
                        tform_chunk(wk_sb, ci, ND, KT, 0, ND, hT)

                # attention helpers (defined early so ST(0)/ST(1) can
                # interleave with the V matmuls: ScalarE starts exps while
                # PE computes V)
                def emit_ST(h):
                    j, r = h // 2, (h % 2) * 64
                    u = upool.tile([128, NT, SEQ], BF16, tag="U")
                    for s_i, (s0, sw) in enumerate(TT):
                        for ci, (c, w) in enumerate(CH_T):
                            ps = patt.tile([128, 512], F32, tag="patt",
                                           name=f"pst{ci}")
                            nc.tensor.matmul(ps[:sw, :w],
                                             lhsT=KT[r:r + 64, j, s0:s0 + sw],
                                             rhs=QT[r:r + 64, j, c:c + w],
                                             start=True, stop=True)
                            nc.scalar.activation(out=u[:sw, s_i, c:c + w],
                                                 in_=ps[:sw, :w],
                                                 func=AF.Exp, scale=SCALE)
                    return u

                def emit_AV(h, u):
                    j, r = h // 2, (h % 2) * 64
                    po = [pbig.tile([128, w], F32, tag="pbig", name=f"po{ci}")
                          for ci, (c, w) in enumerate(CH_T)]
                    for s_i, (s0, sw) in enumerate(TT):
                        for ci, (c, w) in enumerate(CH_T):
                            nc.tensor.matmul(po[ci][:HD + 1, :],
                                             lhsT=vbuf[:sw, s_i, h, :],
                                             rhs=u[:sw, s_i, c:c + w],
                                             start=(s_i == 0), stop=(s_i == NT - 1))
                    rb = rpool.tile([1, SEQ], F32, tag="rb")
                    for ci, (c, w) in enumerate(CH_T):
                        nc.vector.reciprocal(out=rb[0:1, c:c + w],
                                             in_=po[ci][HD:HD + 1, :])
                    for ci, (c, w) in enumerate(CH_T):
                        pbc = pbig.tile([128, 512], F32, tag="pbig", name=f"pbc{ci}")
                        nc.tensor.matmul(pbc[:HD, :w],
                                         lhsT=ones_sb[0:1, :HD],
                                         rhs=rb[0:1, c:c + w],
                                         start=True, stop=True)
                        nc.vector.tensor_copy(out=OT[r:r + 64, j, c:c + w],
                                              in_=po[ci][:HD, :])
                        nc.vector.tensor_mul(out=OT[r:r + 64, j, c:c + w],
                                             in0=OT[r:r + 64, j, c:c + w],
                                             in1=pbc[:HD, :w])

                def v_group(gi):
                    grp = [g for g in range(gi, min(gi + 2, NT))]
                    psums = {}
                    for t_i in grp:
                        psums[t_i] = [pbig.tile([128, w], F32, tag="pbig",
                                                name=f"psv{t_i}_{ci}")
                                      for ci, (s, w) in enumerate(CH_D)]
                    for k in range(ND):
                        for t_i in grp:
                            t0, tw = TT[t_i]
                            for ci, (s, w) in enumerate(CH_D):
                                nc.tensor.matmul(psums[t_i][ci][:tw, :],
                                                 lhsT=hT[:, k, t0:t0 + tw],
                                                 rhs=wv_sb[:, k, s:s + w],
                                                 start=(k == 0), stop=(k == ND - 1))
                    for t_i in grp:
                        t0, tw = TT[t_i]
                        for ci, (s, w) in enumerate(CH_D):
                            h0, nh = s // HD, w // HD
                            vsrc = psums[t_i][ci][:tw, :].rearrange(
                                "p (h d) -> p h d", h=nh)
                            if (t_i + ci) % 2 == 0:
                                nc.vector.tensor_copy(
                                    out=vbuf[:tw, t_i, h0:h0 + nh, 0:HD], in_=vsrc)
                            else:
                                nc.scalar.copy(
                                    out=vbuf[:tw, t_i, h0:h0 + nh, 0:HD], in_=vsrc)

                # V natural into vbuf, with ST(0)/ST(1) sandwiched between
                # groups so ScalarE exps overlap the V matmuls
                with nc.named_scope("v"):
                    wv_sb = load_w(wv[l], "w", wpool, ND, 0, D)
                    u_hist = {}
                    u_hist[0] = emit_ST(0)
                    v_group(0)
                    u_hist[1] = emit_ST(1)
                    v_group(2)
                    v_group(4)

                with nc.named_scope("attn"):
                    for h in range(2, H):
                        emit_AV(h - 2, u_hist.pop(h - 2))
                        u_hist[h] = emit_ST(h)
                    emit_AV(H - 2, u_hist.pop(H - 2))
                    emit_AV(H - 1, u_hist.pop(H - 1))

                # proj + residual + LN2 (interleaved, chunk-major)
                with nc.named_scope("proj"):
                    pw_sb = load_w(pw[l], "w", wpool, ND, 0, D)
                    pbcol = rows.tile([128, ND], F32, tag="pbc")
                    nc.gpsimd.dma_start(out=pbcol[:],
                                        in_=pb[l].rearrange("(t p) -> p t", p=128))
                    ln2_cols = load_ln_cols(ln2g[l], ln2b[l], "2")
                    tform_resid_ln(pw_sb, ND, ND, OT, pbcol, ln2_cols,
                                   with_stats=True)

                # FFN1: 4 slabs of 6 n-tiles each
                with nc.named_scope("ffn1"):
                    fb1col = rows.tile([128, NF], F32, tag="fb1")
                    nc.gpsimd.dma_start(out=fb1col[:],
                                        in_=fb1[l].rearrange("(t p) -> p t", p=128))
                    for sl in range(4):
                        f1_sb = load_w(fw1[l], "f1", f1pool, ND, sl * D, D)
                        for ci in range(2):
                            tform_chunk(f1_sb, ci, ND, h3T, sl * ND, ND, hT,
                                        bias_col=fb1col, relu=True)

                # FFN2 + residual + next-layer LN1 (interleaved, chunk-major)
                with nc.named_scope("ffn2"):
                    fb2col = rows.tile([128, ND], F32, tag="fb2")
                    nc.gpsimd.dma_start(out=fb2col[:],
                                        in_=fb2[l].rearrange("(t p) -> p t", p=128))
                    if l < L - 1:
                        ln1_cols = load_ln_cols(ln1g[l + 1], ln1b[l + 1], "1")
                        tform_resid_ln(f2_sb, ND, NF, h3T, fb2col, ln1_cols,
                                       with_stats=True)
                    else:
                        tform_resid_ln(f2_sb, ND, NF, h3T, fb2col, None,
                                       with_stats=False)

            # ---------- output: cls residual row (row 588 = j4, p76) ----------
            nc.sync.dma_start(out=clsout[:, :], in_=x[76:77, 4, :])

    nc.finalize()
    return nc


# ======================= host side =======================

def _sincos_pos(T, d):
    i = np.arange(T, dtype=np.float64)[:, None]
    j = np.arange(d, dtype=np.float64)[None, :]
    je = np.where(j % 2 == 0, j, j - 1)
    ang = i / np.power(10000.0, je / d)
    pe = np.where(j % 2 == 0, np.sin(ang), np.cos(ang))
    return pe.astype(np.float32)


def _patchify_stacked(img):
    b = img.shape[0]
    x = img.reshape(b, IMG // P, P, IMG // P, P, 3, HS)
    x = x.transpose(0, 1, 3, 6, 2, 4, 5)
    return x.reshape(b, NP * HS, P * P * 3)


def _patchify3(img):
    b = img.shape[0]
    x = img.reshape(b, IMG // P, P, IMG // P, P, 3)
    x = x.transpose(0, 1, 3, 2, 4, 5)
    return x.reshape(b, NP, P * P * 3)


def _layernorm_np(v, g, b, eps=1e-5):
    m = v.mean(axis=-1, keepdims=True)
    s = v.var(axis=-1, keepdims=True)
    return (v - m) / np.sqrt(s + eps) * g + b


PERM = np.concatenate([np.arange(2, 394), np.arange(471, 667),
                       np.array([0, 1]), np.arange(394, 471)])


def kernel(**inputs):
    global LAST_EXEC_NS
    f32 = lambda k: np.asarray(inputs[k], dtype=np.float32)
    bf = lambda a: np.ascontiguousarray(np.asarray(a, dtype=np.float32)
                                        .astype(ml_dtypes.bfloat16))

    if "nc" not in _CACHE:
        _CACHE["nc"] = build_nc()
    nc = _CACHE["nc"]

    images = f32("images")
    goal_imgs = f32("goal_imgs")
    pose = f32("pose")
    txt = np.asarray(inputs["goals_txt"]).astype(np.int64)
    tok_emb = f32("tok_emb")

    # pose MLP (host, exact fp32 – 4.7 MFLOP)
    pose_tok = np.maximum(pose @ f32("pose_w1") + f32("pose_b1"), 0.0) \
        @ f32("pose_w2") + f32("pose_b2")                       # [B, D]

    pos = _sincos_pos(SEQ, D)                                    # [667, D]
    content = np.zeros((B, SEQ, D), np.float32)
    content[:, 0, :] = f32("cls_tok")[0, 0]
    content[:, 1, :] = pose_tok
    content[:, 2:394, :] = f32("obs_b")
    content[:, 394:471, :] = tok_emb[txt]
    content[:, 471:667, :] = f32("goal_b")
    base = (content + pos[None])[:, PERM, :]                     # permuted
    base_pad = np.zeros((B, TPAD, D), np.float32)
    base_pad[:, :SEQ, :] = base

    p_obs = _patchify_stacked(images)                            # [B, 392, 768]
    p_goal = _patchify3(goal_imgs)                               # [B, 196, 768]
    pobsT = bf(p_obs.transpose(0, 2, 1))                         # [B, 768, 392]
    pgoalT_np = np.zeros((B, D, 204), np.float32)
    pgoalT_np[:, :, 8:] = p_goal.transpose(0, 2, 1)
    pgoalT = bf(pgoalT_np)

    shared = {
        "obs_w": bf(f32("obs_w")), "goal_w": bf(f32("goal_w")),
        "wq": bf(f32("wq")), "wk": bf(f32("wk")), "wv": bf(f32("wv")),
        "pw": bf(f32("proj_w")), "fw1": bf(f32("ff_w1")), "fw2": bf(f32("ff_w2")),
        "pb": f32("proj_b"), "fb1": f32("ff_b1"), "fb2": f32("ff_b2"),
        "ln1g": f32("ln1_g"), "ln1b": f32("ln1_b"),
        "ln2g": f32("ln2_g"), "ln2b": f32("ln2_b"),
    }
    in_maps = []
    for b in range(B):
        m = dict(shared)
        m["base"] = np.ascontiguousarray(base_pad[b])
        m["pobsT"] = np.ascontiguousarray(pobsT[b])
        m["pgoalT"] = np.ascontiguousarray(pgoalT[b])
        in_maps.append(m)

    res = run_bass_kernel_spmd(nc, in_maps, list(range(B)), trace=TRACE,
                               trace_cores=TRACE_CORES if TRACE else None)
    LAST_EXEC_NS = res.exec_time_ns

    cls = np.stack([np.asarray(res.results[b]["clsout"][0], np.float32)
                    for b in range(B)])                          # [B, D]
    h = _layernorm_np(cls, f32("lnf_g"), f32("lnf_b"))
    h = _layernorm_np(h, f32("hln_g"), f32("hln_b"))
    out = h @ f32("head_w") + f32("head_b")
    return out.astype(np.float32)


# revision 35
# speedup vs baseline: 1.0519x; 1.0077x over previous
"""Trainium2 Bass kernel: ViT-style multimodal transformer (12L, D=768, H=12).

Strategy: pure data parallel — 8 batch elements, one per NeuronCore.
Each core runs the full transformer on its [667, 768] token sequence.

Device layouts (per core):
  - residual x:   SBUF [128, 6, 768] fp32, token t = j*128 + p  (natural: t on partitions)
  - hT/QT/KT/OT:  SBUF [128, 6, 667] bf16, feature-major (transposed: d on partitions,
                  t on free dim) — the layout matmul wants for both lhsT and rhs roles.
  - attention:    S^T[s,t] = K_h Q_h^T computed per head with s on partitions, exp on
                  ScalarE (no max subtraction; logits are tiny), denominator obtained by
                  appending a ones-column to V in the AV matmul, normalization applied to
                  O' via a K=1 broadcast matmul + multiply.
  - all linear layers except V run in T-form (weights stationary, tokens streamed), so
    every weight block is DMA'd exactly once per layer via the HWDGE rings; proj/ffn2
    T-form outputs are transposed back on PE and accumulated into x, with LN statistics
    (bn_stats) computed eagerly per token tile inside the add-back.
Token order is permuted (attention is permutation-equivariant; positional embeddings are
baked into the additive base): [obs(392) | goal(196) | cls | pose | text(77)], so patch
embeddings land partition-aligned. cls lives at row 588 = (j=4, p=76).
"""

import numpy as np
import ml_dtypes

import concourse.bass as bass
import concourse.bacc as bacc_mod
import concourse.mybir as mybir
import concourse.tile as tile
from concourse.bass_utils import run_bass_kernel_spmd
from concourse.masks import make_identity

BF16 = mybir.dt.bfloat16
F32 = mybir.dt.float32
AF = mybir.ActivationFunctionType
ALU = mybir.AluOpType

L, H, D, HD = 12, 12, 768, 64
P, IMG, NP, HS = 16, 224, 196, 2
TBLK, VOCAB, POSE_DIM, OUT = 77, 96, 7, 7
B = 8
SEQ = 667          # 1 cls + 1 pose + 392 obs + 77 text + 196 goal
TPAD = 768         # padded token slots (6 partition tiles)
NT = 6             # token partition tiles
ND = 6             # feature partition tiles (768/128)
NF = 24            # ffn feature tiles (3072/128)
SCALE = float(D) ** -0.5
EPS = 1e-5

# token tiles (start, width)
TT = [(0, 128), (128, 128), (256, 128), (384, 128), (512, 128), (640, 27)]


def _chunks(total, cap=512):
    s = 0
    out = []
    while s < total:
        w = min(cap, total - s)
        out.append((s, w))
        s += w
    return out


CH_T = _chunks(SEQ)    # [(0,512),(512,155)]
CH_D = _chunks(D)      # [(0,512),(512,256)]

# Runtime knobs (test.py may flip these)
TRACE = False
TRACE_CORES = [0]
LAST_EXEC_NS = None
_CACHE = {}


def build_nc():
    nc = bacc_mod.Bacc()

    # ---- per-core data inputs ----
    base = nc.declare_dram_parameter("base", [TPAD, D], F32, isOutput=False)
    pobsT = nc.declare_dram_parameter("pobsT", [D, 392], BF16, isOutput=False)
    pgoalT = nc.declare_dram_parameter("pgoalT", [D, 204], BF16, isOutput=False)
    # ---- shared weights ----
    obs_w = nc.declare_dram_parameter("obs_w", [D, D], BF16, isOutput=False)
    goal_w = nc.declare_dram_parameter("goal_w", [D, D], BF16, isOutput=False)
    wq = nc.declare_dram_parameter("wq", [L, D, D], BF16, isOutput=False)
    wk = nc.declare_dram_parameter("wk", [L, D, D], BF16, isOutput=False)
    wv = nc.declare_dram_parameter("wv", [L, D, D], BF16, isOutput=False)
    pw = nc.declare_dram_parameter("pw", [L, D, D], BF16, isOutput=False)
    fw1 = nc.declare_dram_parameter("fw1", [L, D, 4 * D], BF16, isOutput=False)
    fw2 = nc.declare_dram_parameter("fw2", [L, 4 * D, D], BF16, isOutput=False)
    pb = nc.declare_dram_parameter("pb", [L, D], F32, isOutput=False)
    fb1 = nc.declare_dram_parameter("fb1", [L, 4 * D], F32, isOutput=False)
    fb2 = nc.declare_dram_parameter("fb2", [L, D], F32, isOutput=False)
    ln1g = nc.declare_dram_parameter("ln1g", [L, D], F32, isOutput=False)
    ln1b = nc.declare_dram_parameter("ln1b", [L, D], F32, isOutput=False)
    ln2g = nc.declare_dram_parameter("ln2g", [L, D], F32, isOutput=False)
    ln2b = nc.declare_dram_parameter("ln2b", [L, D], F32, isOutput=False)
    clsout = nc.declare_dram_parameter("clsout", [1, D], F32, isOutput=True)

    with tile.TileContext(nc) as tc:
        with (
            tc.tile_pool(name="singles", bufs=1) as singles,
            tc.tile_pool(name="lncols", bufs=4) as lncols,
            tc.tile_pool(name="wpool", bufs=2) as wpool,
            tc.tile_pool(name="f1pool", bufs=2) as f1pool,
            tc.tile_pool(name="f2pool", bufs=1) as f2pool,
            tc.tile_pool(name="epool", bufs=3) as epool,
            tc.tile_pool(name="rows", bufs=2) as rows,
            tc.tile_pool(name="hn", bufs=3) as hn,
            tc.tile_pool(name="upool", bufs=2) as upool,
            tc.tile_pool(name="stats", bufs=6) as stats,
            tc.tile_pool(name="rpool", bufs=2) as rpool,
            tc.tile_pool(name="pbig", bufs=4, space="PSUM") as pbig,
            tc.tile_pool(name="patt", bufs=4, space="PSUM") as patt,
        ):
            # ---------- persistent SBUF ----------
            ident = singles.tile([128, 128], BF16)
            make_identity(nc, ident)
            eps_sb = singles.tile([128, 1], F32)
            nc.vector.memset(eps_sb, EPS)
            ones_sb = singles.tile([1, 128], F32)
            nc.vector.memset(ones_sb, 1.0)

            x = singles.tile([128, NT, D], F32)            # residual stream
            hT = singles.tile([128, ND, SEQ], BF16)        # LN output, transposed
            QT = singles.tile([128, ND, SEQ], BF16)
            KT = singles.tile([128, ND, SEQ], BF16)
            vbuf = singles.tile([128, NT, H, HD + 1], BF16)  # V natural + ones col
            OT = singles.tile([128, ND, SEQ], BF16)        # attn out, transposed
            h3T = singles.tile([128, NF, SEQ], BF16)       # relu ffn hidden, transposed
            yT = singles.tile([128, ND, SEQ], BF16)        # proj/ffn2 out, transposed
            st_all = singles.tile([128, NT, 3, 6], F32)    # bn_stats staging
            mv_all = singles.tile([128, NT, 2], F32)       # mean/var per token tile

            nc.vector.memset(vbuf[:, :, :, HD:HD + 1], 1.0)

            # ---------- load residual base ----------
            nc.sync.dma_start(out=x[:], in_=base.rearrange("(j p) d -> p j d", p=128))

            # ---------- helpers ----------
            def emit_stats(ti, tw):
                xi = x[:tw, ti, :].rearrange("p (s c) -> p s c", s=3)
                for s in range(3):
                    nc.vector.bn_stats(out=st_all[:tw, ti, s, :], in_=xi[:, s, :])
                nc.vector.bn_aggr(out=mv_all[:tw, ti, :], in_=st_all[:tw, ti])

            # ---------- patch embeddings ----------
            def embed_add(psrcT, src_w, wtag, ptiles, dests):
                psrc = epool.tile([128, ND, psrcT.shape[1]], BF16, tag=f"p{wtag}",
                                  bufs=1)
                nc.sync.dma_start(out=psrc[:],
                                  in_=psrcT.rearrange("(kt kp) t -> kp kt t", kp=128))
                # whole embed weight matrix in one DMA (f1pool slots are idle
                # until layer-0 ffn1)
                ew = f1pool.tile([128, ND, D], BF16, tag="f1")
                nc.sync.dma_start(
                    out=ew[:], in_=src_w.rearrange("(kt kp) n -> kp kt n", kp=128))
                for gi in range(0, len(ptiles), 2):
                    grp = list(range(gi, min(gi + 2, len(ptiles))))
                    psums = {}
                    for t_i in grp:
                        psums[t_i] = [pbig.tile([128, w], F32, tag="pbig",
                                                name=f"ps{t_i}_{ci}")
                                      for ci, (s, w) in enumerate(CH_D)]
                    for k in range(ND):
                        for t_i in grp:
                            c0, cw = ptiles[t_i]
                            for ci, (s, w) in enumerate(CH_D):
                                nc.tensor.matmul(
                                    psums[t_i][ci][:cw, :],
                                    lhsT=psrc[:, k, c0:c0 + cw],
                                    rhs=ew[:, k, s:s + w],
                                    start=(k == 0), stop=(k == ND - 1))
                    for t_i in grp:
                        c0, cw = ptiles[t_i]
                        r0, xj = dests[t_i]
                        for ci, (s, w) in enumerate(CH_D):
                            nc.vector.tensor_add(out=x[r0:r0 + cw, xj, s:s + w],
                                                 in0=x[r0:r0 + cw, xj, s:s + w],
                                                 in1=psums[t_i][ci][:cw, :])

            embed_add(pobsT, obs_w, "o",
                      [(0, 128), (128, 128), (256, 128), (384, 8)],
                      [(0, 0), (0, 1), (0, 2), (0, 3)])
            embed_add(pgoalT, goal_w, "g",
                      [(0, 128), (128, 76)],
                      [(0, 3), (0, 4)])
            for ti, (t0, tw) in enumerate(TT):
                emit_stats(ti, tw)

            CH_TILES = [[0, 1, 2, 3], [4, 5]]

            def load_ln_cols(g_dram, b_dram, tag):
                gcol = lncols.tile([128, ND], F32, tag=f"g{tag}")
                bcol = lncols.tile([128, ND], F32, tag=f"b{tag}")
                nc.gpsimd.dma_start(out=gcol[:],
                                    in_=g_dram.rearrange("(t p) -> p t", p=128))
                nc.gpsimd.dma_start(out=bcol[:],
                                    in_=b_dram.rearrange("(t p) -> p t", p=128))
                return gcol, bcol

            def ln_tiles(tiles, gcol, bcol):
                """x -> hT for the given token tiles (stats already in mv_all)."""
                for ti in tiles:
                    t0, tw = TT[ti]
                    rstd = stats.tile([128, 1], F32, tag="rstd")
                    nc.scalar.activation(out=rstd[:tw], in_=mv_all[:tw, ti, 1:2],
                                         func=AF.Sqrt, bias=eps_sb[:tw], scale=1.0)
                    nc.vector.reciprocal(out=rstd[:tw], in_=rstd[:tw])
                    hnat = hn.tile([128, D], BF16, tag="hnat")
                    nc.vector.tensor_scalar(out=hnat[:tw], in0=x[:tw, ti, :],
                                            scalar1=mv_all[:tw, ti, 0:1],
                                            scalar2=rstd[:tw],
                                            op0=ALU.subtract, op1=ALU.mult)
                    pt6 = patt.tile([128, ND, 128], BF16, tag="patt")
                    for dj in range(ND):
                        nc.tensor.transpose(pt6[:, dj, :tw],
                                            hnat[:tw, dj * 128:(dj + 1) * 128],
                                            ident[:tw, :tw])
                    for dj in range(ND):
                        nc.scalar.activation(out=hT[:, dj, t0:t0 + tw],
                                             in_=pt6[:, dj, :tw], func=AF.Identity,
                                             scale=gcol[:, dj:dj + 1],
                                             bias=bcol[:, dj:dj + 1])

            def tform_chunk(w_sb, ci, n_cnt, out_sb, on0, k_cnt, src_sb,
                            bias_col=None, relu=False, act_drains=False):
                c0, cw = CH_T[ci]
                for j in range(n_cnt):
                    seg = pbig.tile([128, cw], F32, tag="pbig", name=f"sg{ci}_{j}")
                    for k in range(k_cnt):
                        nc.tensor.matmul(seg[:, :],
                                         lhsT=w_sb[:, k, j * 128:(j + 1) * 128],
                                         rhs=src_sb[:, k, c0:c0 + cw],
                                         start=(k == 0), stop=(k == k_cnt - 1))
                    n_out = on0 + j
                    dst = out_sb[:, n_out, c0:c0 + cw]
                    on_act = act_drains or (j + ci) % 2 == 0
                    if relu:
                        if on_act:
                            nc.scalar.activation(out=dst, in_=seg[:, :], func=AF.Relu,
                                                 bias=bias_col[:, n_out:n_out + 1],
                                                 scale=1.0)
                        else:
                            nc.vector.tensor_scalar(out=dst, in0=seg[:, :],
                                                    scalar1=bias_col[:, n_out:n_out + 1],
                                                    scalar2=0.0,
                                                    op0=ALU.add, op1=ALU.max)
                    elif bias_col is not None:
                        if on_act:
                            nc.scalar.activation(out=dst, in_=seg[:, :],
                                                 func=AF.Identity,
                                                 bias=bias_col[:, n_out:n_out + 1],
                                                 scale=1.0)
                        else:
                            nc.vector.tensor_scalar(out=dst, in0=seg[:, :],
                                                    scalar1=bias_col[:, n_out:n_out + 1],
                                                    scalar2=None, op0=ALU.add)
                    else:
                        if on_act:
                            nc.scalar.copy(out=dst, in_=seg[:, :])
                        else:
                            nc.vector.tensor_copy(out=dst, in_=seg[:, :])

            def resid_chunk(src_sb, ci, with_stats):
                """x += transpose(src_sb) for the token tiles of chunk ci."""
                for ti in CH_TILES[ci]:
                    t0, tw = TT[ti]
                    pt6 = patt.tile([128, D], BF16, tag="patt")
                    for dj in range(ND):
                        nc.tensor.transpose(pt6[:tw, dj * 128:(dj + 1) * 128],
                                            src_sb[:, dj, t0:t0 + tw], ident)
                    nc.vector.tensor_add(out=x[:tw, ti, :], in0=x[:tw, ti, :],
                                         in1=pt6[:tw, :])
                    if with_stats:
                        emit_stats(ti, tw)

            def tform_resid_ln(w_sb, n_cnt, k_cnt, src_sb, bias_col,
                               ln_cols, with_stats, act_drains=True):
                """T-form linear -> yT, chunk-major; residual add-back and the
                following LN interleaved so their DVE/ACT chains hide under
                the other chunk's matmuls. ln_cols None => skip LN emission."""
                tform_chunk(w_sb, 0, n_cnt, yT, 0, k_cnt, src_sb, bias_col,
                            act_drains=act_drains)
                resid_chunk(yT, 0, with_stats)
                tform_chunk(w_sb, 1, n_cnt, yT, 0, k_cnt, src_sb, bias_col,
                            act_drains=act_drains)
                if ln_cols is not None:
                    ln_tiles(CH_TILES[0], *ln_cols)
                resid_chunk(yT, 1, with_stats)
                if ln_cols is not None:
                    ln_tiles(CH_TILES[1], *ln_cols)

            def load_w(w_dram, tag, pool, ksz, c0, cw):
                t = pool.tile([128, ksz, cw], BF16, tag=tag)
                nc.sync.dma_start(
                    out=t[:],
                    in_=w_dram.rearrange("(kt kp) n -> kp kt n", kp=128)[:, :, c0:c0 + cw])
                return t

            # ---------- transformer layers ----------
            ln1_cols = load_ln_cols(ln1g[0], ln1b[0], "1")
            ln_tiles(CH_TILES[0] + CH_TILES[1], *ln1_cols)
            for l in range(L):
                # prefetch the big ffn2 slab early (its buffer frees at the
                # end of the previous layer's ffn2)
                f2_sb = load_w(fw2[l], "f2", f2pool, NF, 0, D)

                with nc.named_scope("qk"):
                    wq_sb = load_w(wq[l], "w", wpool, ND, 0, D)
                    for ci in range(2):
                        tform_chunk(wq_sb, ci, ND, QT, 0, ND, hT)
                    wk_sb = load_w(wk[l], "w", wpool, ND, 0, D)
                    for ci in range(2):
                        tform_chunk(wk_sb, ci, ND, KT, 0, ND, hT)

                # attention helpers (defined early so ST(0)/ST(1) can
                # interleave with the V matmuls: ScalarE starts exps while
                # PE computes V)
                def emit_ST(h):
                    j, r = h // 2, (h % 2) * 64
                    u = upool.tile([128, NT, SEQ], BF16, tag="U")
                    for s_i, (s0, sw) in enumerate(TT):
                        for ci, (c, w) in enumerate(CH_T):
                            ps = patt.tile([128, 512], F32, tag="patt",
                                           name=f"pst{ci}")
                            nc.tensor.matmul(ps[:sw, :w],
                                             lhsT=KT[r:r + 64, j, s0:s0 + sw],
                                             rhs=QT[r:r + 64, j, c:c + w],
                                             start=True, stop=True)
                            nc.scalar.activation(out=u[:sw, s_i, c:c + w],
                                                 in_=ps[:sw, :w],
                                                 func=AF.Exp, scale=SCALE)
                    return u

                def emit_AV(h, u):
                    j, r = h // 2, (h % 2) * 64
                    po = [pbig.tile([128, w], F32, tag="pbig", name=f"po{ci}")
                          for ci, (c, w) in enumerate(CH_T)]
                    for s_i, (s0, sw) in enumerate(TT):
                        for ci, (c, w) in enumerate(CH_T):
                            nc.tensor.matmul(po[ci][:HD + 1, :],
                                             lhsT=vbuf[:sw, s_i, h, :],
                                             rhs=u[:sw, s_i, c:c + w],
                                             start=(s_i == 0), stop=(s_i == NT - 1))
                    rb = rpool.tile([1, SEQ], F32, tag="rb")
                    for ci, (c, w) in enumerate(CH_T):
                        nc.vector.reciprocal(out=rb[0:1, c:c + w],
                                             in_=po[ci][HD:HD + 1, :])
                    for ci, (c, w) in enumerate(CH_T):
                        pbc = pbig.tile([128, 512], F32, tag="pbig", name=f"pbc{ci}")
                        nc.tensor.matmul(pbc[:HD, :w],
                                         lhsT=ones_sb[0:1, :HD],
                                         rhs=rb[0:1, c:c + w],
                                         start=True, stop=True)
                        nc.vector.tensor_copy(out=OT[r:r + 64, j, c:c + w],
                                              in_=po[ci][:HD, :])
                        nc.vector.tensor_mul(out=OT[r:r + 64, j, c:c + w],
                                             in0=OT[r:r + 64, j, c:c + w],
                                             in1=pbc[:HD, :w])

                def v_group(gi):
                    grp = [g for g in range(gi, min(gi + 2, NT))]
                    psums = {}
                    for t_i in grp:
                        psums[t_i] = [pbig.tile([128, w], F32, tag="pbig",
                                                name=f"psv{t_i}_{ci}")
                                      for ci, (s, w) in enumerate(CH_D)]
                    for k in range(ND):
                        for t_i in grp:
                            t0, tw = TT[t_i]
                            for ci, (s, w) in enumerate(CH_D):
                                nc.tensor.matmul(psums[t_i][ci][:tw, :],
                                                 lhsT=hT[:, k, t0:t0 + tw],
                                                 rhs=wv_sb[:, k, s:s + w],
                                                 start=(k == 0), stop=(k == ND - 1))
                    for t_i in grp:
                        t0, tw = TT[t_i]
                        for ci, (s, w) in enumerate(CH_D):
                            h0, nh = s // HD, w // HD
                            vsrc = psums[t_i][ci][:tw, :].rearrange(
                                "p (h d) -> p h d", h=nh)
                            if (t_i + ci) % 2 == 0:
                                nc.vector.tensor_copy(
                                    out=vbuf[:tw, t_i, h0:h0 + nh, 0:HD], in_=vsrc)
                            else:
                                nc.scalar.copy(
                                    out=vbuf[:tw, t_i, h0:h0 + nh, 0:HD], in_=vsrc)

                # V natural into vbuf, with ST(0)/ST(1) sandwiched between
                # groups so ScalarE exps overlap the V matmuls
                with nc.named_scope("v"):
                    wv_sb = load_w(wv[l], "w", wpool, ND, 0, D)
                    u_hist = {}
                    u_hist[0] = emit_ST(0)
                    v_group(0)
                    u_hist[1] = emit_ST(1)
                    v_group(2)
                    v_group(4)

                with nc.named_scope("attn"):
                    for h in range(2, H):
                        emit_AV(h - 2, u_hist.pop(h - 2))
                        u_hist[h] = emit_ST(h)
                    emit_AV(H - 2, u_hist.pop(H - 2))
                    emit_AV(H - 1, u_hist.pop(H - 1))

                # proj + residual + LN2 (interleaved, chunk-major)
                with nc.named_scope("proj"):
                    pw_sb = load_w(pw[l], "w", wpool, ND, 0, D)
                    pbcol = rows.tile([128, ND], F32, tag="pbc")
                    nc.gpsimd.dma_start(out=pbcol[:],
                                        in_=pb[l].rearrange("(t p) -> p t", p=128))
                    ln2_cols = load_ln_cols(ln2g[l], ln2b[l], "2")
                    tform_resid_ln(pw_sb, ND, ND, OT, pbcol, ln2_cols,
                                   with_stats=True)

                # FFN1: 4 slabs of 6 n-tiles each
                with nc.named_scope("ffn1"):
                    fb1col = rows.tile([128, NF], F32, tag="fb1")
                    nc.gpsimd.dma_start(out=fb1col[:],
                                        in_=fb1[l].rearrange("(t p) -> p t", p=128))
                    for sl in range(4):
                        f1_sb = load_w(fw1[l], "f1", f1pool, ND, sl * D, D)
                        for ci in range(2):
                            tform_chunk(f1_sb, ci, ND, h3T, sl * ND, ND, hT,
                                        bias_col=fb1col, relu=True)

                # FFN2 + residual + next-layer LN1 (interleaved, chunk-major)
                with nc.named_scope("ffn2"):
                    fb2col = rows.tile([128, ND], F32, tag="fb2")
                    nc.gpsimd.dma_start(out=fb2col[:],
                                        in_=fb2[l].rearrange("(t p) -> p t", p=128))
                    if l < L - 1:
                        ln1_cols = load_ln_cols(ln1g[l + 1], ln1b[l + 1], "1")
                        tform_resid_ln(f2_sb, ND, NF, h3T, fb2col, ln1_cols,
                                       with_stats=True)
                    else:
                        tform_resid_ln(f2_sb, ND, NF, h3T, fb2col, None,
                                       with_stats=False)

            # ---------- output: cls residual row (row 588 = j4, p76) ----------
            nc.sync.dma_start(out=clsout[:, :], in_=x[76:77, 4, :])

    nc.finalize()
    return nc


# ======================= host side =======================

def _sincos_pos(T, d):
    i = np.arange(T, dtype=np.float64)[:, None]
    j = np.arange(d, dtype=np.float64)[None, :]
    je = np.where(j % 2 == 0, j, j - 1)
    ang = i / np.power(10000.0, je / d)
    pe = np.where(j % 2 == 0, np.sin(ang), np.cos(ang))
    return pe.astype(np.float32)


def _patchify_stacked(img):
    b = img.shape[0]
    x = img.reshape(b, IMG // P, P, IMG // P, P, 3, HS)
    x = x.transpose(0, 1, 3, 6, 2, 4, 5)
    return x.reshape(b, NP * HS, P * P * 3)


def _patchify3(img):
    b = img.shape[0]
    x = img.reshape(b, IMG // P, P, IMG // P, P, 3)
    x = x.transpose(0, 1, 3, 2, 4, 5)
    return x.reshape(b, NP, P * P * 3)


def _layernorm_np(v, g, b, eps=1e-5):
    m = v.mean(axis=-1, keepdims=True)
    s = v.var(axis=-1, keepdims=True)
    return (v - m) / np.sqrt(s + eps) * g + b


PERM = np.concatenate([np.arange(2, 394), np.arange(471, 667),
                       np.array([0, 1]), np.arange(394, 471)])


def kernel(**inputs):
    global LAST_EXEC_NS
    f32 = lambda k: np.asarray(inputs[k], dtype=np.float32)
    bf = lambda a: np.ascontiguousarray(np.asarray(a, dtype=np.float32)
                                        .astype(ml_dtypes.bfloat16))

    if "nc" not in _CACHE:
        _CACHE["nc"] = build_nc()
    nc = _CACHE["nc"]

    images = f32("images")
    goal_imgs = f32("goal_imgs")
    pose = f32("pose")
    txt = np.asarray(inputs["goals_txt"]).astype(np.int64)
    tok_emb = f32("tok_emb")

    # pose MLP (host, exact fp32 – 4.7 MFLOP)
    pose_tok = np.maximum(pose @ f32("pose_w1") + f32("pose_b1"), 0.0) \
        @ f32("pose_w2") + f32("pose_b2")                       # [B, D]

    pos = _sincos_pos(SEQ, D)                                    # [667, D]
    content = np.zeros((B, SEQ, D), np.float32)
    content[:, 0, :] = f32("cls_tok")[0, 0]
    content[:, 1, :] = pose_tok
    content[:, 2:394, :] = f32("obs_b")
    content[:, 394:471, :] = tok_emb[txt]
    content[:, 471:667, :] = f32("goal_b")
    base = (content + pos[None])[:, PERM, :]                     # permuted
    base_pad = np.zeros((B, TPAD, D), np.float32)
    base_pad[:, :SEQ, :] = base

    p_obs = _patchify_stacked(images)                            # [B, 392, 768]
    p_goal = _patchify3(goal_imgs)                               # [B, 196, 768]
    pobsT = bf(p_obs.transpose(0, 2, 1))                         # [B, 768, 392]
    pgoalT_np = np.zeros((B, D, 204), np.float32)
    pgoalT_np[:, :, 8:] = p_goal.transpose(0, 2, 1)
    pgoalT = bf(pgoalT_np)

    shared = {
        "obs_w": bf(f32("obs_w")), "goal_w": bf(f32("goal_w")),
        "wq": bf(f32("wq")), "wk": bf(f32("wk")), "wv": bf(f32("wv")),
        "pw": bf(f32("proj_w")), "fw1": bf(f32("ff_w1")), "fw2": bf(f32("ff_w2")),
        "pb": f32("proj_b"), "fb1": f32("ff_b1"), "fb2": f32("ff_b2"),
        "ln1g": f32("ln1_g"), "ln1b": f32("ln1_b"),
        "ln2g": f32("ln2_g"), "ln2b": f32("ln2_b"),
    }
    in_maps = []
    for b in range(B):
        m = dict(shared)
        m["base"] = np.ascontiguousarray(base_pad[b])
        m["pobsT"] = np.ascontiguousarray(pobsT[b])
        m["pgoalT"] = np.ascontiguousarray(pgoalT[b])
        in_maps.append(m)

    res = run_bass_kernel_spmd(nc, in_maps, list(range(B)), trace=TRACE,
                               trace_cores=TRACE_CORES if TRACE else None)
    LAST_EXEC_NS = res.exec_time_ns

    cls = np.stack([np.asarray(res.results[b]["clsout"][0], np.float32)
                    for b in range(B)])                          # [B, D]
    h = _layernorm_np(cls, f32("lnf_g"), f32("lnf_b"))
    h = _layernorm_np(h, f32("hln_g"), f32("hln_b"))
    out = h @ f32("head_w") + f32("head_b")
    return out.astype(np.float32)
